# revision 86
# baseline (speedup 1.0000x reference)
"""Trainium2 Bass kernel for nn_Attention_54254026883778.

Single-head attention with an additive post-softmax intensity term:
    q/k/v = X @ W{q,k,v}.T + b;  scores = q k^T / sqrt(D)
    attn  = softmax(scores) + intensity;  out = (attn @ v) @ Wo.T + bo

Sharding: 8 cores = 4 batches x 2 sequence halves. Each core computes
K^T and V for its own 1024 rows; the partner half arrives via 2-rank
AllGathers through shared DRAM (global t-order, SPMD-static).

Precision plan: Q/K projections and the scores matmul run in fp8e4m3
with DoubleRow (2x PE throughput). Host ships X and 32*W{q,k}^T in fp8
(the x32 shift keeps the tiny weights in fp8's normal range); the exp
activation folds the 1/(32*32) back in on top of 1/sqrt(D). Softmax
errors are ~1000x attenuated in the output (softmax rows sum to 1 while
the additive intensity rows sum to ~1024), so fp8 there is safe. The
V projection, PV and output projection stay bf16.

Dataflow per core (own 1024 query rows, full 2048 keys):
    warmup   dummy matmuls under the initial DMA so the PE HAM
             clock-gate opens before real work arrives
    K^T      [dout | t-own]  fp8 DR, j-outer; per-j fp8 AllGather
    V        [t-own | dv]    bf16, dv-half-outer; per-half AllGather
    Q^T      [dout | s]      fp8 DR
    scores   [s | t]         fp8 DR -> exp on ACT with fused
        row-accumulate -> 1/den on DVE -> diag(recip) ->
        attn^T tile = E-slice.T @ diag(recip)  (PE transposes+normalizes)
        -> DVE adds intensity^T while draining PSUM -> attn^T [t | s]
    out^T    [dv | s]    = V-chunk.T @ attn^T   (bf16)
    final^T  [do | s]    = WoT-chunk.T @ out^T  -> DRAM, host transposes
Biases: q/k (x32) and o enter as per-partition adds during PSUM extract;
v enters as a rank-1 fp32r matmul bv (x) (1 + rowsum(I)).

DMA plan (the previous round stalled 50us on queue head-blocking):
  - big inputs are host-swizzled to [P, chunks, cols] so each is ONE
    trigger with 8-16KB contiguous per-partition packets
  - all collective staging/unpacks ride the SWDGE (gpsimd) queue in an
    emission order that never parks an unpack in front of a stage
  - intensity loads + WO + outputs stay on the sync HWDGE queue
"""

import numpy as np
import ml_dtypes

P = 128
D = 1024
S = 2048          # keys per batch (full sequence)
SH = 1024         # query rows owned by each core
DC = D // P       # 8  contraction chunks over model dim
TC = S // P       # 16 t (key) chunks
NT = 512          # matmul moving free dim / psum bank
SJ = SH // NT     # 2  s-tiles of own rows
TJ = S // NT      # 4  t-tiles
SCALE = 1.0 / 32.0        # 1/sqrt(D)
SCALE8 = SCALE / 1024.0   # undo the x32 on q and k

_CACHE = {}


def _build_module():
    import concourse.bass as bass
    import concourse.tile as tile
    import concourse.mybir as mybir
    from concourse import bacc
    from concourse.masks import make_identity

    f32 = mybir.dt.float32
    f32r = mybir.dt.float32r
    bf16 = mybir.dt.bfloat16
    fp8 = mybir.dt.float8e4
    DR = mybir.MatmulPerfMode.DoubleRow
    Exp = mybir.ActivationFunctionType.Exp
    add = mybir.AluOpType.add
    mult = mybir.AluOpType.mult

    nc = bacc.Bacc("TRN2", target_bir_lowering=False, debug=False,
                   num_devices=8)

    X8_d = nc.dram_tensor("X8T", [P, DC, SH], fp8, kind="ExternalInput")
    XQ_d = nc.dram_tensor("XQT", [P, DC, SH], bf16, kind="ExternalInput")
    WQ_d = nc.dram_tensor("WQ8", [P, DC, D], fp8, kind="ExternalInput")
    WK_d = nc.dram_tensor("WK8", [P, DC, D], fp8, kind="ExternalInput")
    WV_d = nc.dram_tensor("WVT", [P, DC, D], bf16, kind="ExternalInput")
    WO_d = nc.dram_tensor("WOT", [P, DC, D], bf16, kind="ExternalInput")
    BCOL_d = nc.dram_tensor("BCOL", [P, 3 * DC], f32, kind="ExternalInput")
    BROW_d = nc.dram_tensor("BROW", [1, D + SH], f32, kind="ExternalInput")
    IT_d = nc.dram_tensor("IT", [P, SJ, TC, NT], bf16, kind="ExternalInput")
    OUT_d = nc.dram_tensor("OUTT", [D, SH], f32, kind="ExternalOutput")

    out_v = OUT_d[:].rearrange("(c p) s -> p c s", p=P)

    GROUPS = [[0, 1], [2, 3], [4, 5], [6, 7]]
    NDR = DC // 2  # 4 contraction pair-chunks for DoubleRow

    with tile.TileContext(nc) as tc:
        with (
            tc.tile_pool(name="persist", bufs=1) as persist,
            tc.tile_pool(name="mm_ps", bufs=5, space="PSUM") as mm_ps,
            tc.tile_pool(name="den_ps", bufs=2, space="PSUM") as den_ps,
            tc.tile_pool(name="rb_ps", bufs=1, space="PSUM") as rb_ps,
            tc.tile_pool(name="dram", bufs=1, space="DRAM") as dram_pool,
        ):
            # ---- persistent tiles -------------------------------------
            KT_sb = persist.tile([P, 2, SJ, DC, NT], fp8)   # K^T gathered
            V_sb = persist.tile([P, 2, TC, NT], bf16)       # V [t | half,dv]
            QT_sb = persist.tile([P, DC, SH], fp8, tag="qt_ot")
            X8_sb = persist.tile([P, DC, SH], fp8)          # X^T own, fp8
            XQ_sb = persist.tile([P, DC, SH], bf16, tag="xq_at")
            KL_sb = persist.tile([P, SJ, DC, NT], fp8)      # K^T local (x32)
            VL_sb = persist.tile([P, SJ, DC, NT], bf16, tag="vl_it")
            # intensity^T shares VL's region: its 4MB DMA then WAR-gates on
            # the V staging reads (~80us) — a REAL dependency the scheduler
            # cannot hoist past, keeping IT out of the startup HBM rush.
            IT_sb = persist.tile([P, SJ, TC, NT], bf16, tag="vl_it")
            AT_sb = persist.tile([P, TC, SH], bf16, tag="xq_at")  # attn^T

            ident = persist.tile([P, P], bf16)
            make_identity(nc, ident)
            WARM_sb = persist.tile([P, NT], bf16)
            nc.vector.memset(WARM_sb[:], 0.0)
            # ones column (bf16) for the den row-matmuls, ones row (f32r)
            # for the recip broadcast outer product
            ONESB = persist.tile([P, 1], bf16)
            nc.vector.memset(ONESB[:], 1.0)
            ONESR = persist.tile([1, P], f32r)
            with tc.tile_pool(name="ones_pool", bufs=1) as ones_pool:
                ones_f = ones_pool.tile([1, P], f32)
                nc.vector.memset(ones_f[:], 1.0)
                nc.vector.tensor_copy(out=ONESR[:], in_=ones_f[:])
            # bq*32|bk*32|bo as per-partition columns, added on PSUM extract
            BCOL_sb = persist.tile([P, 3 * DC], f32)
            nc.sync.dma_start(BCOL_sb[:], BCOL_d[:])
            # bv and the attn rowsums feed the PV rank-1 bias matmul; fp32r
            # operands must come from a rounding instruction, so stage the
            # DMA through a DVE copy.
            BROW_r = persist.tile([1, D + SH], f32r)
            with tc.tile_pool(name="brow_pool", bufs=1) as brow_pool:
                BROW_ld = brow_pool.tile([1, D + SH], f32)
                nc.sync.dma_start(BROW_ld[:], BROW_d[:])
                nc.vector.tensor_copy(out=BROW_r[:], in_=BROW_ld[:])
            BV_sb = BROW_r[0:1, 0:D]
            RS_sb = BROW_r[0:1, D:D + SH]                   # 1 + rowsum(I)

            # ---- PE warmup: dummy matmuls under the initial DMA -------
            for _ in range(20):
                wps = mm_ps.tile([P, NT], f32, tag="mm", name="warm")
                nc.tensor.matmul(wps[:], ident[:], WARM_sb[:],
                                 start=True, stop=True)


            # collective DRAM tiles: K per j-half (fp8), V per dv-half
            k_in = [dram_pool.tile([P, DC, NT], fp8, name=f"k_in{j}")
                    for j in range(SJ)]
            k_out = [dram_pool.tile([2, P, DC, NT], fp8, name=f"k_out{j}")
                     for j in range(SJ)]
            v_in = [dram_pool.tile([P, DC, NT], bf16, name=f"v_in{h}")
                    for h in range(2)]
            v_out = [dram_pool.tile([2, P, DC, NT], bf16, name=f"v_out{h}")
                     for h in range(2)]

            # ---- phase A: K full-local, V-local/gather, Q -------------
            with (
                tc.tile_pool(name="w8pool", bufs=1) as w8pool,
                tc.tile_pool(name="wvpool", bufs=1) as wvpool,
            ):
                # All inputs ride the sync queue in deadline order (the
                # proven config); WQ8 shares the single w8pool buffer with
                # WK8 so its load is WAR-gated until K consumed the weights.
                # Inputs in deadline order on the sync queue. (Artificial
                # WAW-memset gating does NOT work: the list scheduler
                # hoists dependency-free memsets to t=0, releasing the
                # gated loads into the startup HBM rush.)
                WK_sb = w8pool.tile([P, DC, D], fp8, tag="w8")
                WV_sb = wvpool.tile([P, DC, D], bf16)
                nc.sync.dma_start(X8_sb[:], X8_d[:])
                nc.sync.dma_start(WK_sb[:], WK_d[:])
                nc.sync.dma_start(XQ_sb[:], XQ_d[:])
                nc.sync.dma_start(WV_sb[:], WV_d[:])

                # K^T local [dout, t-own] fp8 DoubleRow, j-outer; stage and
                # gather each j-half on the SWDGE as soon as it is done so
                # both doorbells ring by ~36us and the wire work finishes
                # well before the Q projection does.
                for j in range(SJ):
                    for c in range(DC):
                        ps = mm_ps.tile([P, NT], f32, tag="mm", name="ps")
                        for i in range(NDR):
                            nc.tensor.matmul(
                                ps[:],
                                WK_sb[:, 2 * i:2 * i + 2, c * P:(c + 1) * P],
                                X8_sb[:, 2 * i:2 * i + 2,
                                      j * NT:(j + 1) * NT],
                                start=(i == 0), stop=(i == NDR - 1),
                                perf_mode=DR,
                            )
                        nc.vector.tensor_scalar_add(
                            KL_sb[:, j, c, :], ps[:],
                            BCOL_sb[:, DC + c:DC + c + 1])
                    nc.gpsimd.dma_start(k_in[j][:], KL_sb[:, j])
                    nc.gpsimd.collective_compute(
                        "AllGather", mybir.AluOpType.bypass,
                        replica_groups=GROUPS,
                        ins=[k_in[j].opt()], outs=[k_out[j].opt()])

                # V local, dv-half-outer (h = output half); VL is [P,h,t,dv]
                # so each half stages as one contiguous 8KB/partition DMA.
                # The SECOND half runs AFTER the Q projection: it fills the
                # PE while the K AllGather results land, so the scores never
                # wait on the collective chain (and either order of {V-h1,
                # Q} keeps that property if the scheduler swaps them).
                def v_half(h):
                    for t in range(DC):
                        ps = mm_ps.tile([P, NT], f32, tag="mm", name="ps")
                        for dc in range(DC):
                            nc.tensor.matmul(
                                ps[:],
                                XQ_sb[:, dc, t * P:(t + 1) * P],
                                WV_sb[:, dc, h * NT:(h + 1) * NT],
                                start=(dc == 0),
                                stop=(dc == DC - 1),
                            )
                        nc.vector.tensor_copy(
                            out=VL_sb[:, h, t, :], in_=ps[:])
                    nc.gpsimd.dma_start(v_in[h][:], VL_sb[:, h])
                    nc.gpsimd.collective_compute(
                        "AllGather", mybir.AluOpType.bypass,
                        replica_groups=GROUPS,
                        ins=[v_in[h].opt()], outs=[v_out[h].opt()])

                v_half(0)

                # Q^T [dout, s-own] fp8 DoubleRow; WQ8 reuses WK8's buffer
                # (bufs=1 WAR-gates its DMA behind the last K matmul, which
                # keeps it out of the startup HBM rush but lands ~40us, well
                # before Q starts).
                WQ_sb = w8pool.tile([P, DC, D], fp8, tag="w8")
                nc.sync.dma_start(WQ_sb[:], WQ_d[:])
                # K unpacks on the sync HWDGE queue, emitted after every
                # input trigger so their ccK semaphore waits can only
                # head-block the intensity loads (needed ~25us later)
                for j in range(SJ):
                    for b in range(2):
                        nc.sync.dma_start(KT_sb[:, b, j], k_out[j][b])
                # intensity behind the unpacks: keeps its 4MB off the
                # startup HBM rush; needed only by the normalize passes
                for sj in range(SJ):
                    nc.sync.dma_start(IT_sb[:, sj], IT_d[:, sj])
                for c in range(DC):
                    psl = [mm_ps.tile([P, NT], f32, tag="mm", name="ps")
                           for _ in range(SJ)]
                    for i in range(NDR):
                        for j in range(SJ):
                            nc.tensor.matmul(
                                psl[j][:],
                                WQ_sb[:, 2 * i:2 * i + 2, c * P:(c + 1) * P],
                                X8_sb[:, 2 * i:2 * i + 2,
                                      j * NT:(j + 1) * NT],
                                start=(i == 0), stop=(i == NDR - 1),
                                perf_mode=DR,
                            )
                    for j in range(SJ):
                        nc.vector.tensor_scalar_add(
                            QT_sb[:, c, j * NT:(j + 1) * NT], psl[j][:],
                            BCOL_sb[:, c:c + 1])

                v_half(1)
                # V unpacks last on the SWDGE queue (PV needs them latest)
                for h in range(2):
                    for b in range(2):
                        nc.gpsimd.dma_start(
                            V_sb[:, h, b * DC:(b + 1) * DC, :], v_out[h][b])

            # ---- phase B: scores^T -> exp -> den -> normalize -> +I^T -
            # scores are computed DIRECTLY transposed ([t | s] tiles via
            # lhsT = K^T chunk, rhs = Q^T), which kills the 128 transpose
            # matmuls AND lets each tile depend on only one K gather half:
            # the j0-gather tiles run first so the j1 gather lands in their
            # shadow. Denominators are ones-vector matmuls accumulated in
            # PSUM (sum over the t partition axis), the reciprocal row is
            # broadcast to 128 partitions with a rank-1 f32r outer product,
            # and normalize + intensity-add are two 512-wide DVE ops/tile.
            # tiles needing only gather-j0 vs gather-j1:
            TT_J = [[0, 1, 2, 3, 8, 9, 10, 11], [4, 5, 6, 7, 12, 13, 14, 15]]
            with (
                tc.tile_pool(name="rec_pool", bufs=2) as rec_pool,
                tc.tile_pool(name="rbs_pool", bufs=2) as rbs_pool,
                tc.tile_pool(name="wo_pool", bufs=1) as wo_pool,
                tc.tile_pool(name="fin_pool", bufs=3) as fin_pool,
            ):
                def sj_scores(sj, jp):
                    for tt in TT_J[jp]:
                        blk, j, nl = tt // 8, (tt % 8) // 4, tt % 4
                        ps = mm_ps.tile([P, NT], f32, tag="mm", name="ps")
                        for i in range(NDR):
                            nc.tensor.matmul(
                                ps[:],
                                KT_sb[:, blk, j, 2 * i:2 * i + 2,
                                      nl * P:(nl + 1) * P],
                                QT_sb[:, 2 * i:2 * i + 2,
                                      sj * NT:(sj + 1) * NT],
                                start=(i == 0), stop=(i == NDR - 1),
                                perf_mode=DR,
                            )
                        nc.scalar.activation(
                            AT_sb[:, tt, sj * NT:(sj + 1) * NT], ps[:], Exp,
                            scale=SCALE8)

                def sj_den(sj):
                    dps = den_ps.tile([1, NT], f32, tag="den")
                    for n in range(TC):
                        nc.tensor.matmul(
                            dps[0:1, :], ONESB[:, 0:1],
                            AT_sb[:, n, sj * NT:(sj + 1) * NT],
                            start=(n == 0), stop=(n == TC - 1),
                        )
                    rec = rec_pool.tile([1, NT], f32r, tag="rec")
                    with nc.allow_low_precision(
                            reason="f32r reciprocal row for the rank-1 "
                                   "broadcast; 22-bit mantissa is ample"):
                        nc.vector.reciprocal(rec[0:1, :], dps[0:1, :])
                    rb = rb_ps.tile([P, NT], f32, tag="rb")
                    nc.tensor.matmul(rb[:], ONESR[0:1, :], rec[0:1, :],
                                     start=True, stop=True)
                    RB_sb = rbs_pool.tile([P, NT], bf16, tag="rbs")
                    nc.vector.tensor_copy(out=RB_sb[:], in_=rb[:])
                    return RB_sb

                def sj_normalize(sj, RB_sb):
                    with nc.allow_low_precision(
                            reason="bf16 in-place normalize+intensity add; "
                                   "matches the prior bf16 attn precision"):
                        for tt in range(TC):
                            sl = AT_sb[:, tt, sj * NT:(sj + 1) * NT]
                            nc.vector.tensor_tensor(sl, sl, RB_sb[:], mult)
                            nc.vector.tensor_tensor(
                                sl, sl, IT_sb[:, sj, tt, :], add)

                # ---- phase D helpers: PV -> out^T -> projection --------
                OT_sb = persist.tile([P, DC, SH], bf16, tag="qt_ot")
                WO_sb = wo_pool.tile([P, DC, D], bf16)
                nc.sync.dma_start(WO_sb[:], WO_d[:])

                def pv_half(sj):
                    for dvi in range(DC):
                        ps = mm_ps.tile([P, NT], f32, tag="mm")
                        for t in range(TC):
                            nc.tensor.matmul(
                                ps[:],
                                V_sb[:, dvi // 4, t,
                                     (dvi % 4) * P:(dvi % 4 + 1) * P],
                                AT_sb[:, t, sj * NT:(sj + 1) * NT],
                                start=(t == 0),
                                stop=False,
                            )
                        # bias: bv (x) (1 + rowsum(I))
                        nc.tensor.matmul(
                            ps[:], BV_sb[0:1, dvi * P:(dvi + 1) * P],
                            RS_sb[0:1, sj * NT:(sj + 1) * NT],
                            start=False, stop=True)
                        nc.vector.tensor_copy(
                            out=OT_sb[:, dvi, sj * NT:(sj + 1) * NT],
                            in_=ps[:])

                def proj_half(sj):
                    for doi in range(DC):
                        ps = mm_ps.tile([P, NT], f32, tag="mm")
                        for dvc in range(DC):
                            nc.tensor.matmul(
                                ps[:],
                                WO_sb[:, dvc, doi * P:(doi + 1) * P],
                                OT_sb[:, dvc, sj * NT:(sj + 1) * NT],
                                start=(dvc == 0), stop=(dvc == DC - 1),
                            )
                        F_sb = fin_pool.tile([P, NT], f32, tag="fin")
                        nc.vector.tensor_scalar_add(
                            F_sb[:], ps[:],
                            BCOL_sb[:, 2 * DC + doi:2 * DC + doi + 1])
                        nc.sync.dma_start(
                            out_v[:, doi, sj * NT:(sj + 1) * NT], F_sb[:])

                # ---- emission order: every serial cross-engine chain is
                # covered by real matmuls. Both s-halves' gather-j0 tiles
                # run first (j1 lands in their shadow); den0/broadcast slot
                # between the j1 tile groups; PV of s-half 0 hides norm0's
                # DVE tail and den1; norm1 hides under den1+proj0.
                sj_scores(0, 0)
                sj_scores(1, 0)
                sj_scores(0, 1)
                rb0 = sj_den(0)
                sj_scores(1, 1)
                sj_normalize(0, rb0)
                rb1 = sj_den(1)
                sj_normalize(1, rb1)
                pv_half(0)
                proj_half(0)
                pv_half(1)
                proj_half(1)

    nc.compile()
    return nc


def _get_module():
    if "nc" not in _CACHE:
        _CACHE["nc"] = _build_module()
    return _CACHE["nc"]


def _chunked(a, ncols):
    """[D, ncols] -> [P, DC, ncols] with partition p holding rows c*128+p."""
    return np.ascontiguousarray(
        a.reshape(DC, P, ncols).transpose(1, 0, 2))


def _make_in_maps(inputs):
    X = np.asarray(inputs["X"], dtype=np.float32)
    intensity = np.asarray(inputs["intensity"], dtype=np.float32)
    bf = ml_dtypes.bfloat16
    f8 = ml_dtypes.float8_e4m3
    Wq8 = _chunked(np.ascontiguousarray(
        np.asarray(inputs["Wq"], np.float32).T * 32.0), D).astype(f8)
    Wk8 = _chunked(np.ascontiguousarray(
        np.asarray(inputs["Wk"], np.float32).T * 32.0), D).astype(f8)
    WvT = _chunked(np.ascontiguousarray(
        np.asarray(inputs["Wv"], np.float32).T), D).astype(bf)
    WoT = _chunked(np.ascontiguousarray(
        np.asarray(inputs["Wo"], np.float32).T), D).astype(bf)
    bq, bk, bv, bo = (np.asarray(inputs[k], np.float32).reshape(D)
                      for k in ("bq", "bk", "bv", "bo"))
    BCOL = np.concatenate(
        [b.reshape(DC, P).T for b in (32.0 * bq, 32.0 * bk, bo)], axis=1
    ).astype(np.float32)  # [128, 24]

    in_maps = []
    for c in range(8):
        b, h = c // 2, c % 2
        XT = np.ascontiguousarray(X[b, h * SH:(h + 1) * SH, :].T)
        XQT = _chunked(XT, SH).astype(bf)
        X8T = _chunked(XT, SH).astype(f8)
        Islc = intensity[b, h * SH:(h + 1) * SH, :]
        # I^T chunked sj-major: IT[p, sj, tt, n] = I^T[tt*128+p, sj*512+n],
        # matching the directly-transposed attn tiles; each sj-half loads
        # as one contiguous 16KB/partition DMA
        IT = np.ascontiguousarray(
            Islc.T.reshape(TC, P, SJ, NT).transpose(1, 2, 0, 3)).astype(bf)
        rows = 1.0 + Islc.sum(axis=1, dtype=np.float64).astype(np.float32)
        BROW = np.concatenate([bv, rows]).reshape(1, D + SH)
        in_maps.append({
            "X8T": X8T, "XQT": XQT, "WQ8": Wq8, "WK8": Wk8,
            "WVT": WvT, "WOT": WoT,
            "BCOL": BCOL, "BROW": BROW, "IT": IT,
        })
    return in_maps


def _gather(results):
    out = np.empty((4, S, D), dtype=np.float32)
    for c in range(8):
        b, h = c // 2, c % 2
        out[b, h * SH:(h + 1) * SH, :] = results[c]["OUTT"].T
    return out


def kernel(**inputs):
    from concourse import bass_utils

    in_maps = _make_in_maps(inputs)
    nc = _get_module()
    res = bass_utils.run_bass_kernel_spmd(nc, in_maps, core_ids=list(range(8)))
    return _gather(res.results)


# revision 90
# speedup vs baseline: 1.0299x; 1.0299x over previous
"""Trainium2 Bass kernel for nn_Attention_54254026883778.

Single-head attention with an additive post-softmax intensity term:
    q/k/v = X @ W{q,k,v}.T + b;  scores = q k^T / sqrt(D)
    attn  = softmax(scores) + intensity;  out = (attn @ v) @ Wo.T + bo

Sharding: 8 cores = 4 batches x 2 sequence halves. Each core computes
K^T and V for its own 1024 rows; the partner half arrives via 2-rank
AllGathers through shared DRAM (global t-order, SPMD-static).

Precision plan: Q/K projections and the scores matmul run in fp8e4m3
with DoubleRow (2x PE throughput). Host ships X and 32*W{q,k}^T in fp8
(the x32 shift keeps the tiny weights in fp8's normal range); the exp
activation folds the 1/(32*32) back in on top of 1/sqrt(D). Softmax
errors are ~1000x attenuated in the output (softmax rows sum to 1 while
the additive intensity rows sum to ~1024), so fp8 there is safe. The
V projection, PV and output projection stay bf16.

Dataflow per core (own 1024 query rows, full 2048 keys):
    warmup   dummy matmuls under the initial DMA so the PE HAM
             clock-gate opens before real work arrives
    K^T      [dout | t-own]  fp8 DR, j-outer; per-j fp8 AllGather
    V        [t-own | dv]    bf16, dv-half-outer; per-half AllGather
    Q^T      [dout | s]      fp8 DR
    scores   [s | t]         fp8 DR -> exp on ACT with fused
        row-accumulate -> 1/den on DVE -> diag(recip) ->
        attn^T tile = E-slice.T @ diag(recip)  (PE transposes+normalizes)
        -> DVE adds intensity^T while draining PSUM -> attn^T [t | s]
    out^T    [dv | s]    = V-chunk.T @ attn^T   (bf16)
    final^T  [do | s]    = WoT-chunk.T @ out^T  -> DRAM, host transposes
Biases: q/k (x32) and o enter as per-partition adds during PSUM extract;
v enters as a rank-1 fp32r matmul bv (x) (1 + rowsum(I)).

DMA plan (the previous round stalled 50us on queue head-blocking):
  - big inputs are host-swizzled to [P, chunks, cols] so each is ONE
    trigger with 8-16KB contiguous per-partition packets
  - all collective staging/unpacks ride the SWDGE (gpsimd) queue in an
    emission order that never parks an unpack in front of a stage
  - intensity loads + WO + outputs stay on the sync HWDGE queue
"""

import numpy as np
import ml_dtypes

P = 128
D = 1024
S = 2048          # keys per batch (full sequence)
SH = 1024         # query rows owned by each core
DC = D // P       # 8  contraction chunks over model dim
TC = S // P       # 16 t (key) chunks
NT = 512          # matmul moving free dim / psum bank
SJ = SH // NT     # 2  s-tiles of own rows
TJ = S // NT      # 4  t-tiles
SCALE = 1.0 / 32.0        # 1/sqrt(D)
SCALE8 = SCALE / 1024.0   # undo the x32 on q and k

_CACHE = {}


def _build_module():
    import concourse.bass as bass
    import concourse.tile as tile
    import concourse.mybir as mybir
    from concourse import bacc
    from concourse.masks import make_identity

    f32 = mybir.dt.float32
    f32r = mybir.dt.float32r
    bf16 = mybir.dt.bfloat16
    fp8 = mybir.dt.float8e4
    DR = mybir.MatmulPerfMode.DoubleRow
    Exp = mybir.ActivationFunctionType.Exp
    add = mybir.AluOpType.add
    mult = mybir.AluOpType.mult

    nc = bacc.Bacc("TRN2", target_bir_lowering=False, debug=False,
                   num_devices=8)

    X8_d = nc.dram_tensor("X8T", [P, DC, SH], fp8, kind="ExternalInput")
    XQ_d = nc.dram_tensor("XQT", [P, DC, SH], bf16, kind="ExternalInput")
    WQ_d = nc.dram_tensor("WQ8", [P, DC, D], fp8, kind="ExternalInput")
    WK_d = nc.dram_tensor("WK8", [P, DC, D], fp8, kind="ExternalInput")
    WV_d = nc.dram_tensor("WVT", [P, DC, D], bf16, kind="ExternalInput")
    WO_d = nc.dram_tensor("WOT", [P, DC, D], bf16, kind="ExternalInput")
    BCOL_d = nc.dram_tensor("BCOL", [P, 3 * DC], f32, kind="ExternalInput")
    BROW_d = nc.dram_tensor("BROW", [1, D + SH], f32, kind="ExternalInput")
    IT_d = nc.dram_tensor("IT", [P, SJ, TC, NT], bf16, kind="ExternalInput")
    OUT_d = nc.dram_tensor("OUTT", [D, SH], f32, kind="ExternalOutput")

    out_v = OUT_d[:].rearrange("(c p) s -> p c s", p=P)

    GROUPS = [[0, 1], [2, 3], [4, 5], [6, 7]]
    NDR = DC // 2  # 4 contraction pair-chunks for DoubleRow

    with tile.TileContext(nc) as tc:
        with (
            tc.tile_pool(name="persist", bufs=1) as persist,
            tc.tile_pool(name="mm_ps", bufs=5, space="PSUM") as mm_ps,
            tc.tile_pool(name="den_ps", bufs=2, space="PSUM") as den_ps,
            tc.tile_pool(name="rb_ps", bufs=1, space="PSUM") as rb_ps,
            tc.tile_pool(name="dram", bufs=1, space="DRAM") as dram_pool,
        ):
            # ---- persistent tiles -------------------------------------
            # Load-sequencing via REAL aliasing dependencies (the only
            # scheduler-proof mechanism): the PE-warmup matmuls read scratch
            # buffers that share regions with XQ/WV, so those 4MB of DMAs
            # WAR-gate on warmup-end (~= when K's own 2MB of inputs are in)
            # instead of joining the t=0 HBM rush; intensity shares KL's
            # region so its 4MB waits for the K staging reads (~37us).
            KT_sb = persist.tile([P, 2, SJ, DC, NT], fp8)   # K^T gathered
            V_sb = persist.tile([P, 2, TC, NT], bf16)       # V [t | half,dv]
            QT_sb = persist.tile([P, DC, SH], fp8, tag="qt_ot")
            X8_sb = persist.tile([P, DC, SH], fp8)          # X^T own, fp8
            WARM_A = persist.tile([P, DC, SH], bf16, tag="xq_at")
            XQ_sb = persist.tile([P, DC, SH], bf16, tag="xq_at")
            KL_sb = persist.tile([P, SJ, DC, NT], fp8, tag="kl_it")
            IT_sb = persist.tile([P, SJ, TC, NT], bf16, tag="kl_it")
            VL_sb = persist.tile([P, SJ, DC, NT], bf16)     # local V halves
            AT_sb = persist.tile([P, TC, SH], bf16, tag="xq_at")  # attn^T
            WARM_B = persist.tile([P, DC, D], bf16, tag="wv_w")
            WV_sb = persist.tile([P, DC, D], bf16, tag="wv_w")

            ident = persist.tile([P, P], bf16)
            make_identity(nc, ident)
            nc.vector.memset(WARM_A[:, 0, 0:NT], 0.0)
            nc.vector.memset(WARM_B[:, 0, 0:NT], 0.0)
            # ones column (bf16) for the den row-matmuls, ones row (f32r)
            # for the recip broadcast outer product
            ONESB = persist.tile([P, 1], bf16)
            nc.vector.memset(ONESB[:], 1.0)
            ONESR = persist.tile([1, P], f32r)
            with tc.tile_pool(name="ones_pool", bufs=1) as ones_pool:
                ones_f = ones_pool.tile([1, P], f32)
                nc.vector.memset(ones_f[:], 1.0)
                nc.vector.tensor_copy(out=ONESR[:], in_=ones_f[:])
            # bq*32|bk*32|bo as per-partition columns, added on PSUM extract
            BCOL_sb = persist.tile([P, 3 * DC], f32)
            nc.sync.dma_start(BCOL_sb[:], BCOL_d[:])
            # bv and the attn rowsums feed the PV rank-1 bias matmul; fp32r
            # operands must come from a rounding instruction, so stage the
            # DMA through a DVE copy.
            BROW_r = persist.tile([1, D + SH], f32r)
            with tc.tile_pool(name="brow_pool", bufs=1) as brow_pool:
                BROW_ld = brow_pool.tile([1, D + SH], f32)
                nc.sync.dma_start(BROW_ld[:], BROW_d[:])
                nc.vector.tensor_copy(out=BROW_r[:], in_=BROW_ld[:])
            BV_sb = BROW_r[0:1, 0:D]
            RS_sb = BROW_r[0:1, D:D + SH]                   # 1 + rowsum(I)

            # ---- PE warmup: dummy matmuls under the initial DMA; their
            # scratch reads double as the XQ/WV release gates ----------
            for i in range(8):
                wps = mm_ps.tile([P, NT], f32, tag="mm", name="warm")
                wsrc = WARM_A if i % 2 == 0 else WARM_B
                nc.tensor.matmul(wps[:], ident[:], wsrc[:, 0, 0:NT],
                                 start=True, stop=True)


            # collective DRAM tiles: K per j-half (fp8), V per dv-half
            k_in = [dram_pool.tile([P, DC, NT], fp8, name=f"k_in{j}")
                    for j in range(SJ)]
            k_out = [dram_pool.tile([2, P, DC, NT], fp8, name=f"k_out{j}")
                     for j in range(SJ)]
            v_in = [dram_pool.tile([P, DC, NT], bf16, name=f"v_in{h}")
                    for h in range(2)]
            v_out = [dram_pool.tile([2, P, DC, NT], bf16, name=f"v_out{h}")
                     for h in range(2)]

            # ---- phase A: K-local/gather, V-local/gather, Q -----------
            with tc.tile_pool(name="w8pool", bufs=1) as w8pool:
                # All inputs ride the sync queue; WQ8 shares the single
                # w8pool buffer with WK8 so its load is WAR-gated until K
                # consumed the weights, and XQ/WV/IT are alias-gated above,
                # so only X8+WK8 (2MB) contend for HBM at t=0.
                WK_sb = w8pool.tile([P, DC, D], fp8, tag="w8")
                nc.sync.dma_start(X8_sb[:], X8_d[:])
                nc.sync.dma_start(WK_sb[:], WK_d[:])
                nc.sync.dma_start(XQ_sb[:], XQ_d[:])
                nc.sync.dma_start(WV_sb[:], WV_d[:])

                # K^T local [dout, t-own] fp8 DoubleRow, j-outer; stage and
                # gather each j-half on the SWDGE as soon as it is done so
                # both doorbells ring by ~36us and the wire work finishes
                # well before the Q projection does.
                for j in range(SJ):
                    for c in range(DC):
                        ps = mm_ps.tile([P, NT], f32, tag="mm", name="ps")
                        for i in range(NDR):
                            nc.tensor.matmul(
                                ps[:],
                                WK_sb[:, 2 * i:2 * i + 2, c * P:(c + 1) * P],
                                X8_sb[:, 2 * i:2 * i + 2,
                                      j * NT:(j + 1) * NT],
                                start=(i == 0), stop=(i == NDR - 1),
                                perf_mode=DR,
                            )
                        nc.vector.tensor_scalar_add(
                            KL_sb[:, j, c, :], ps[:],
                            BCOL_sb[:, DC + c:DC + c + 1])
                    nc.gpsimd.dma_start(k_in[j][:], KL_sb[:, j])
                    nc.gpsimd.collective_compute(
                        "AllGather", mybir.AluOpType.bypass,
                        replica_groups=GROUPS,
                        ins=[k_in[j].opt()], outs=[k_out[j].opt()])

                # V local, dv-half-outer (h = output half); VL is [P,h,t,dv]
                # so each half stages as one contiguous 8KB/partition DMA.
                # The SECOND half runs AFTER the Q projection: it fills the
                # PE while the K AllGather results land, so the scores never
                # wait on the collective chain (and either order of {V-h1,
                # Q} keeps that property if the scheduler swaps them).
                def v_half(h):
                    for t in range(DC):
                        ps = mm_ps.tile([P, NT], f32, tag="mm", name="ps")
                        for dc in range(DC):
                            nc.tensor.matmul(
                                ps[:],
                                XQ_sb[:, dc, t * P:(t + 1) * P],
                                WV_sb[:, dc, h * NT:(h + 1) * NT],
                                start=(dc == 0),
                                stop=(dc == DC - 1),
                            )
                        nc.vector.tensor_copy(
                            out=VL_sb[:, h, t, :], in_=ps[:])
                    nc.gpsimd.dma_start(v_in[h][:], VL_sb[:, h])
                    nc.gpsimd.collective_compute(
                        "AllGather", mybir.AluOpType.bypass,
                        replica_groups=GROUPS,
                        ins=[v_in[h].opt()], outs=[v_out[h].opt()])

                v_half(0)

                # Q^T [dout, s-own] fp8 DoubleRow; WQ8 reuses WK8's buffer
                # (bufs=1 WAR-gates its DMA behind the last K matmul, which
                # keeps it out of the startup HBM rush but lands ~40us, well
                # before Q starts).
                WQ_sb = w8pool.tile([P, DC, D], fp8, tag="w8")
                nc.sync.dma_start(WQ_sb[:], WQ_d[:])
                # K unpacks on the sync HWDGE queue, emitted after every
                # input trigger so their ccK semaphore waits can only
                # head-block the intensity loads (needed ~25us later)
                for j in range(SJ):
                    for b in range(2):
                        nc.sync.dma_start(KT_sb[:, b, j], k_out[j][b])
                # intensity behind the unpacks: keeps its 4MB off the
                # startup HBM rush; needed only by the normalize passes
                for sj in range(SJ):
                    nc.sync.dma_start(IT_sb[:, sj], IT_d[:, sj])
                for c in range(DC):
                    psl = [mm_ps.tile([P, NT], f32, tag="mm", name="ps")
                           for _ in range(SJ)]
                    for i in range(NDR):
                        for j in range(SJ):
                            nc.tensor.matmul(
                                psl[j][:],
                                WQ_sb[:, 2 * i:2 * i + 2, c * P:(c + 1) * P],
                                X8_sb[:, 2 * i:2 * i + 2,
                                      j * NT:(j + 1) * NT],
                                start=(i == 0), stop=(i == NDR - 1),
                                perf_mode=DR,
                            )
                    for j in range(SJ):
                        nc.vector.tensor_scalar_add(
                            QT_sb[:, c, j * NT:(j + 1) * NT], psl[j][:],
                            BCOL_sb[:, c:c + 1])

                v_half(1)
                # V unpacks last on the SWDGE queue (PV needs them latest)
                for h in range(2):
                    for b in range(2):
                        nc.gpsimd.dma_start(
                            V_sb[:, h, b * DC:(b + 1) * DC, :], v_out[h][b])

            # ---- phase B: scores^T -> exp -> den -> normalize -> +I^T -
            # scores are computed DIRECTLY transposed ([t | s] tiles via
            # lhsT = K^T chunk, rhs = Q^T), which kills the 128 transpose
            # matmuls AND lets each tile depend on only one K gather half:
            # the j0-gather tiles run first so the j1 gather lands in their
            # shadow. Denominators are ones-vector matmuls accumulated in
            # PSUM (sum over the t partition axis), the reciprocal row is
            # broadcast to 128 partitions with a rank-1 f32r outer product,
            # and normalize + intensity-add are two 512-wide DVE ops/tile.
            # j0 tiles of both blocks first, then j1 tiles:
            TT_ORDER = [0, 1, 2, 3, 8, 9, 10, 11, 4, 5, 6, 7, 12, 13, 14, 15]
            with (
                tc.tile_pool(name="rec_pool", bufs=2) as rec_pool,
                tc.tile_pool(name="rbs_pool", bufs=2) as rbs_pool,
            ):
                def sj_scores(sj):
                    for tt in TT_ORDER:
                        blk, j, nl = tt // 8, (tt % 8) // 4, tt % 4
                        ps = mm_ps.tile([P, NT], f32, tag="mm", name="ps")
                        for i in range(NDR):
                            nc.tensor.matmul(
                                ps[:],
                                KT_sb[:, blk, j, 2 * i:2 * i + 2,
                                      nl * P:(nl + 1) * P],
                                QT_sb[:, 2 * i:2 * i + 2,
                                      sj * NT:(sj + 1) * NT],
                                start=(i == 0), stop=(i == NDR - 1),
                                perf_mode=DR,
                            )
                        nc.scalar.activation(
                            AT_sb[:, tt, sj * NT:(sj + 1) * NT], ps[:], Exp,
                            scale=SCALE8)

                def sj_den(sj):
                    dps = den_ps.tile([1, NT], f32, tag="den")
                    for n in range(TC):
                        nc.tensor.matmul(
                            dps[0:1, :], ONESB[:, 0:1],
                            AT_sb[:, n, sj * NT:(sj + 1) * NT],
                            start=(n == 0), stop=(n == TC - 1),
                        )
                    rec = rec_pool.tile([1, NT], f32r, tag="rec")
                    with nc.allow_low_precision(
                            reason="f32r reciprocal row for the rank-1 "
                                   "broadcast; 22-bit mantissa is ample"):
                        nc.vector.reciprocal(rec[0:1, :], dps[0:1, :])
                    rb = rb_ps.tile([P, NT], f32, tag="rb")
                    nc.tensor.matmul(rb[:], ONESR[0:1, :], rec[0:1, :],
                                     start=True, stop=True)
                    RB_sb = rbs_pool.tile([P, NT], bf16, tag="rbs")
                    nc.vector.tensor_copy(out=RB_sb[:], in_=rb[:])
                    return RB_sb

                def sj_normalize(sj, RB_sb):
                    with nc.allow_low_precision(
                            reason="bf16 in-place normalize+intensity add; "
                                   "matches the prior bf16 attn precision"):
                        for tt in range(TC):
                            sl = AT_sb[:, tt, sj * NT:(sj + 1) * NT]
                            nc.vector.tensor_tensor(sl, sl, RB_sb[:], mult)
                            nc.vector.tensor_tensor(
                                sl, sl, IT_sb[:, sj, tt, :], add)

                # den/normalize of sj=0 interleave between the two scores
                # halves: norm0 (DVE) hides under sj=1's matmuls, and PV's
                # first s-half later hides norm1.
                sj_scores(0)
                rb0 = sj_den(0)
                sj_normalize(0, rb0)
                sj_scores(1)
                rb1 = sj_den(1)
                sj_normalize(1, rb1)

            # ---- phase D/E: PV -> out^T, then projection per s-tile ---
            OT_sb = persist.tile([P, DC, SH], bf16, tag="qt_ot")
            with (
                tc.tile_pool(name="wo_pool", bufs=1) as wo_pool,
                tc.tile_pool(name="fin_pool", bufs=3) as fin_pool,
            ):
                WO_sb = wo_pool.tile([P, DC, D], bf16)
                nc.sync.dma_start(WO_sb[:], WO_d[:])
                for sj in range(SJ):
                    for dvi in range(DC):
                        ps = mm_ps.tile([P, NT], f32, tag="mm")
                        for t in range(TC):
                            nc.tensor.matmul(
                                ps[:],
                                V_sb[:, dvi // 4, t,
                                     (dvi % 4) * P:(dvi % 4 + 1) * P],
                                AT_sb[:, t, sj * NT:(sj + 1) * NT],
                                start=(t == 0),
                                stop=False,
                            )
                        # bias: bv (x) (1 + rowsum(I))
                        nc.tensor.matmul(
                            ps[:], BV_sb[0:1, dvi * P:(dvi + 1) * P],
                            RS_sb[0:1, sj * NT:(sj + 1) * NT],
                            start=False, stop=True)
                        nc.vector.tensor_copy(
                            out=OT_sb[:, dvi, sj * NT:(sj + 1) * NT], in_=ps[:])

                    for doi in range(DC):
                        ps = mm_ps.tile([P, NT], f32, tag="mm")
                        for dvc in range(DC):
                            nc.tensor.matmul(
                                ps[:],
                                WO_sb[:, dvc, doi * P:(doi + 1) * P],
                                OT_sb[:, dvc, sj * NT:(sj + 1) * NT],
                                start=(dvc == 0), stop=(dvc == DC - 1),
                            )
                        F_sb = fin_pool.tile([P, NT], f32, tag="fin")
                        nc.vector.tensor_scalar_add(
                            F_sb[:], ps[:],
                            BCOL_sb[:, 2 * DC + doi:2 * DC + doi + 1])
                        nc.sync.dma_start(
                            out_v[:, doi, sj * NT:(sj + 1) * NT], F_sb[:])

    nc.compile()
    return nc


def _get_module():
    if "nc" not in _CACHE:
        _CACHE["nc"] = _build_module()
    return _CACHE["nc"]


def _chunked(a, ncols):
    """[D, ncols] -> [P, DC, ncols] with partition p holding rows c*128+p."""
    return np.ascontiguousarray(
        a.reshape(DC, P, ncols).transpose(1, 0, 2))


def _make_in_maps(inputs):
    X = np.asarray(inputs["X"], dtype=np.float32)
    intensity = np.asarray(inputs["intensity"], dtype=np.float32)
    bf = ml_dtypes.bfloat16
    f8 = ml_dtypes.float8_e4m3
    Wq8 = _chunked(np.ascontiguousarray(
        np.asarray(inputs["Wq"], np.float32).T * 32.0), D).astype(f8)
    Wk8 = _chunked(np.ascontiguousarray(
        np.asarray(inputs["Wk"], np.float32).T * 32.0), D).astype(f8)
    WvT = _chunked(np.ascontiguousarray(
        np.asarray(inputs["Wv"], np.float32).T), D).astype(bf)
    WoT = _chunked(np.ascontiguousarray(
        np.asarray(inputs["Wo"], np.float32).T), D).astype(bf)
    bq, bk, bv, bo = (np.asarray(inputs[k], np.float32).reshape(D)
                      for k in ("bq", "bk", "bv", "bo"))
    BCOL = np.concatenate(
        [b.reshape(DC, P).T for b in (32.0 * bq, 32.0 * bk, bo)], axis=1
    ).astype(np.float32)  # [128, 24]

    in_maps = []
    for c in range(8):
        b, h = c // 2, c % 2
        XT = np.ascontiguousarray(X[b, h * SH:(h + 1) * SH, :].T)
        XQT = _chunked(XT, SH).astype(bf)
        X8T = _chunked(XT, SH).astype(f8)
        Islc = intensity[b, h * SH:(h + 1) * SH, :]
        # I^T chunked sj-major: IT[p, sj, tt, n] = I^T[tt*128+p, sj*512+n],
        # matching the directly-transposed attn tiles; each sj-half loads
        # as one contiguous 16KB/partition DMA
        IT = np.ascontiguousarray(
            Islc.T.reshape(TC, P, SJ, NT).transpose(1, 2, 0, 3)).astype(bf)
        rows = 1.0 + Islc.sum(axis=1, dtype=np.float64).astype(np.float32)
        BROW = np.concatenate([bv, rows]).reshape(1, D + SH)
        in_maps.append({
            "X8T": X8T, "XQT": XQT, "WQ8": Wq8, "WK8": Wk8,
            "WVT": WvT, "WOT": WoT,
            "BCOL": BCOL, "BROW": BROW, "IT": IT,
        })
    return in_maps


def _gather(results):
    out = np.empty((4, S, D), dtype=np.float32)
    for c in range(8):
        b, h = c // 2, c % 2
        out[b, h * SH:(h + 1) * SH, :] = results[c]["OUTT"].T
    return out


def kernel(**inputs):
    from concourse import bass_utils

    in_maps = _make_in_maps(inputs)
    nc = _get_module()
    res = bass_utils.run_bass_kernel_spmd(nc, in_maps, core_ids=list(range(8)))
    return _gather(res.results)


# revision 91
# speedup vs baseline: 1.1211x; 1.0885x over previous
"""Trainium2 Bass kernel for nn_Attention_54254026883778.

Single-head attention with an additive post-softmax intensity term:
    q/k/v = X @ W{q,k,v}.T + b;  scores = q k^T / sqrt(D)
    attn  = softmax(scores) + intensity;  out = (attn @ v) @ Wo.T + bo

Sharding: 8 cores = 4 batches x 2 sequence halves. Each core computes
K^T and V for its own 1024 rows; the partner half arrives via 2-rank
AllGathers through shared DRAM (global t-order, SPMD-static).

Precision plan: Q/K projections and the scores matmul run in fp8e4m3
with DoubleRow (2x PE throughput). Host ships X and 32*W{q,k}^T in fp8
(the x32 shift keeps the tiny weights in fp8's normal range); the exp
activation folds the 1/(32*32) back in on top of 1/sqrt(D). Softmax
errors are ~1000x attenuated in the output (softmax rows sum to 1 while
the additive intensity rows sum to ~1024), so fp8 there is safe. The
V projection, PV and output projection stay bf16.

Dataflow per core (own 1024 query rows, full 2048 keys):
    warmup   dummy matmuls under the initial DMA so the PE HAM
             clock-gate opens before real work arrives
    K^T      [dout | t-own]  fp8 DR, j-outer; per-j fp8 AllGather
    V        [t-own | dv]    bf16, dv-half-outer; per-half AllGather
    Q^T      [dout | s]      fp8 DR
    scores   [s | t]         fp8 DR -> exp on ACT with fused
        row-accumulate -> 1/den on DVE -> diag(recip) ->
        attn^T tile = E-slice.T @ diag(recip)  (PE transposes+normalizes)
        -> DVE adds intensity^T while draining PSUM -> attn^T [t | s]
    out^T    [dv | s]    = V-chunk.T @ attn^T   (bf16)
    final^T  [do | s]    = WoT-chunk.T @ out^T  -> DRAM, host transposes
Biases: q/k (x32) and o enter as per-partition adds during PSUM extract;
v enters as a rank-1 fp32r matmul bv (x) (1 + rowsum(I)).

DMA plan (the previous round stalled 50us on queue head-blocking):
  - big inputs are host-swizzled to [P, chunks, cols] so each is ONE
    trigger with 8-16KB contiguous per-partition packets
  - all collective staging/unpacks ride the SWDGE (gpsimd) queue in an
    emission order that never parks an unpack in front of a stage
  - intensity loads + WO + outputs stay on the sync HWDGE queue
"""

import numpy as np
import ml_dtypes

P = 128
D = 1024
S = 2048          # keys per batch (full sequence)
SH = 1024         # query rows owned by each core
DC = D // P       # 8  contraction chunks over model dim
TC = S // P       # 16 t (key) chunks
NT = 512          # matmul moving free dim / psum bank
SJ = SH // NT     # 2  s-tiles of own rows
TJ = S // NT      # 4  t-tiles
SCALE = 1.0 / 32.0        # 1/sqrt(D)
SCALE8 = SCALE / 1024.0   # undo the x32 on q and k

_CACHE = {}


def _build_module():
    import concourse.bass as bass
    import concourse.tile as tile
    import concourse.mybir as mybir
    from concourse import bacc
    from concourse.masks import make_identity

    f32 = mybir.dt.float32
    f32r = mybir.dt.float32r
    bf16 = mybir.dt.bfloat16
    fp8 = mybir.dt.float8e4
    DR = mybir.MatmulPerfMode.DoubleRow
    Exp = mybir.ActivationFunctionType.Exp
    add = mybir.AluOpType.add
    mult = mybir.AluOpType.mult

    nc = bacc.Bacc("TRN2", target_bir_lowering=False, debug=False,
                   num_devices=8)

    X8_d = nc.dram_tensor("X8T", [P, DC, SH], fp8, kind="ExternalInput")
    XQ_d = nc.dram_tensor("XQT", [P, DC, SH], bf16, kind="ExternalInput")
    WQ_d = nc.dram_tensor("WQ8", [P, DC, D], fp8, kind="ExternalInput")
    WK_d = nc.dram_tensor("WK8", [P, DC, D], fp8, kind="ExternalInput")
    WV_d = nc.dram_tensor("WVT", [P, DC, D], bf16, kind="ExternalInput")
    WO_d = nc.dram_tensor("WOT", [P, DC, D], bf16, kind="ExternalInput")
    BCOL_d = nc.dram_tensor("BCOL", [P, 3 * DC], f32, kind="ExternalInput")
    BROW_d = nc.dram_tensor("BROW", [1, D + SH], f32, kind="ExternalInput")
    IT_d = nc.dram_tensor("IT", [P, SJ, TC, NT], bf16, kind="ExternalInput")
    OUT_d = nc.dram_tensor("OUTT", [D, SH], f32, kind="ExternalOutput")

    out_v = OUT_d[:].rearrange("(c p) s -> p c s", p=P)

    GROUPS = [[0, 1], [2, 3], [4, 5], [6, 7]]
    NDR = DC // 2  # 4 contraction pair-chunks for DoubleRow

    with tile.TileContext(nc) as tc:
        with (
            tc.tile_pool(name="persist", bufs=1) as persist,
            tc.tile_pool(name="mm_ps", bufs=5, space="PSUM") as mm_ps,
            tc.tile_pool(name="den_ps", bufs=2, space="PSUM") as den_ps,
            tc.tile_pool(name="rb_ps", bufs=1, space="PSUM") as rb_ps,
            tc.tile_pool(name="dram", bufs=1, space="DRAM") as dram_pool,
        ):
            # ---- persistent tiles -------------------------------------
            # Load-sequencing via REAL aliasing dependencies (the only
            # scheduler-proof mechanism): the PE-warmup matmuls read scratch
            # buffers that share regions with XQ/WV, so those 4MB of DMAs
            # WAR-gate on warmup-end (~= when K's own 2MB of inputs are in)
            # instead of joining the t=0 HBM rush; intensity shares KL's
            # region so its 4MB waits for the K staging reads (~37us).
            KT_sb = persist.tile([P, 2, SJ, DC, NT], fp8)   # K^T gathered
            V_sb = persist.tile([P, 2, TC, NT], bf16)       # V [t | half,dv]
            QT_sb = persist.tile([P, DC, SH], fp8, tag="qt_ot")
            X8_sb = persist.tile([P, DC, SH], fp8)          # X^T own, fp8
            WARM_A = persist.tile([P, DC, SH], bf16, tag="xq_at")
            XQ_sb = persist.tile([P, DC, SH], bf16, tag="xq_at")
            KL_sb = persist.tile([P, SJ, DC, NT], fp8, tag="kl_it")
            IT_sb = persist.tile([P, SJ, TC, NT], bf16, tag="kl_it")
            VL_sb = persist.tile([P, SJ, DC, NT], bf16)     # local V halves
            AT_sb = persist.tile([P, TC, SH], bf16, tag="xq_at")  # attn^T
            WARM_B = persist.tile([P, DC, D], bf16, tag="wv_w")
            WV_sb = persist.tile([P, DC, D], bf16, tag="wv_w")

            ident = persist.tile([P, P], bf16)
            make_identity(nc, ident)
            nc.vector.memset(WARM_A[:, 0, 0:NT], 0.0)
            nc.vector.memset(WARM_B[:, 0, 0:NT], 0.0)
            # ones column (bf16) for the den row-matmuls, ones row (f32r)
            # for the recip broadcast outer product
            ONESB = persist.tile([P, 1], bf16)
            nc.vector.memset(ONESB[:], 1.0)
            ONESR = persist.tile([1, P], f32r)
            with tc.tile_pool(name="ones_pool", bufs=1) as ones_pool:
                ones_f = ones_pool.tile([1, P], f32)
                nc.vector.memset(ones_f[:], 1.0)
                nc.vector.tensor_copy(out=ONESR[:], in_=ones_f[:])
            # bq*32|bk*32|bo as per-partition columns, added on PSUM extract
            BCOL_sb = persist.tile([P, 3 * DC], f32)
            nc.sync.dma_start(BCOL_sb[:], BCOL_d[:])
            # bv and the attn rowsums feed the PV rank-1 bias matmul; fp32r
            # operands must come from a rounding instruction, so stage the
            # DMA through a DVE copy.
            BROW_r = persist.tile([1, D + SH], f32r)
            with tc.tile_pool(name="brow_pool", bufs=1) as brow_pool:
                BROW_ld = brow_pool.tile([1, D + SH], f32)
                nc.sync.dma_start(BROW_ld[:], BROW_d[:])
                nc.vector.tensor_copy(out=BROW_r[:], in_=BROW_ld[:])
            BV_sb = BROW_r[0:1, 0:D]
            RS_sb = BROW_r[0:1, D:D + SH]                   # 1 + rowsum(I)

            # ---- PE warmup: dummy matmuls under the initial DMA; their
            # scratch reads double as the XQ/WV release gates ----------
            for i in range(14):
                wps = mm_ps.tile([P, NT], f32, tag="mm", name="warm")
                wsrc = WARM_A if i % 2 == 0 else WARM_B
                nc.tensor.matmul(wps[:], ident[:], wsrc[:, 0, 0:NT],
                                 start=True, stop=True)


            # collective DRAM tiles: K per j-half (fp8), V per dv-half
            k_in = [dram_pool.tile([P, DC, NT], fp8, name=f"k_in{j}")
                    for j in range(SJ)]
            k_out = [dram_pool.tile([2, P, DC, NT], fp8, name=f"k_out{j}")
                     for j in range(SJ)]
            v_in = [dram_pool.tile([P, DC, NT], bf16, name=f"v_in{h}")
                    for h in range(2)]
            v_out = [dram_pool.tile([2, P, DC, NT], bf16, name=f"v_out{h}")
                     for h in range(2)]

            # ---- phase A: K-local/gather, V-local/gather, Q -----------
            with tc.tile_pool(name="w8pool", bufs=1) as w8pool:
                # All inputs ride the sync queue; WQ8 shares the single
                # w8pool buffer with WK8 so its load is WAR-gated until K
                # consumed the weights, and XQ/WV/IT are alias-gated above,
                # so only X8+WK8 (2MB) contend for HBM at t=0.
                WK_sb = w8pool.tile([P, DC, D], fp8, tag="w8")
                nc.sync.dma_start(X8_sb[:], X8_d[:])
                nc.sync.dma_start(WK_sb[:], WK_d[:])
                nc.sync.dma_start(XQ_sb[:], XQ_d[:])
                nc.sync.dma_start(WV_sb[:], WV_d[:])

                # K^T local [dout, t-own] fp8 DoubleRow, j-outer; stage and
                # gather each j-half on the SWDGE as soon as it is done so
                # both doorbells ring by ~36us and the wire work finishes
                # well before the Q projection does.
                for j in range(SJ):
                    for c in range(DC):
                        ps = mm_ps.tile([P, NT], f32, tag="mm", name="ps")
                        for i in range(NDR):
                            nc.tensor.matmul(
                                ps[:],
                                WK_sb[:, 2 * i:2 * i + 2, c * P:(c + 1) * P],
                                X8_sb[:, 2 * i:2 * i + 2,
                                      j * NT:(j + 1) * NT],
                                start=(i == 0), stop=(i == NDR - 1),
                                perf_mode=DR,
                            )
                        nc.vector.tensor_scalar_add(
                            KL_sb[:, j, c, :], ps[:],
                            BCOL_sb[:, DC + c:DC + c + 1])
                    nc.gpsimd.dma_start(k_in[j][:], KL_sb[:, j])
                    nc.gpsimd.collective_compute(
                        "AllGather", mybir.AluOpType.bypass,
                        replica_groups=GROUPS,
                        ins=[k_in[j].opt()], outs=[k_out[j].opt()])

                # V local, dv-half-outer (h = output half); VL is [P,h,t,dv]
                # so each half stages as one contiguous 8KB/partition DMA.
                # The SECOND half runs AFTER the Q projection: it fills the
                # PE while the K AllGather results land, so the scores never
                # wait on the collective chain (and either order of {V-h1,
                # Q} keeps that property if the scheduler swaps them).
                def v_half(h):
                    for t in range(DC):
                        ps = mm_ps.tile([P, NT], f32, tag="mm", name="ps")
                        for dc in range(DC):
                            nc.tensor.matmul(
                                ps[:],
                                XQ_sb[:, dc, t * P:(t + 1) * P],
                                WV_sb[:, dc, h * NT:(h + 1) * NT],
                                start=(dc == 0),
                                stop=(dc == DC - 1),
                            )
                        nc.vector.tensor_copy(
                            out=VL_sb[:, h, t, :], in_=ps[:])
                    nc.gpsimd.dma_start(v_in[h][:], VL_sb[:, h])
                    nc.gpsimd.collective_compute(
                        "AllGather", mybir.AluOpType.bypass,
                        replica_groups=GROUPS,
                        ins=[v_in[h].opt()], outs=[v_out[h].opt()])

                v_half(0)

                # Q^T [dout, s-own] fp8 DoubleRow; WQ8 reuses WK8's buffer
                # (bufs=1 WAR-gates its DMA behind the last K matmul, which
                # keeps it out of the startup HBM rush but lands ~40us, well
                # before Q starts).
                WQ_sb = w8pool.tile([P, DC, D], fp8, tag="w8")
                nc.sync.dma_start(WQ_sb[:], WQ_d[:])
                # K unpacks on the sync HWDGE queue, emitted after every
                # input trigger so their ccK semaphore waits can only
                # head-block the intensity loads (needed ~25us later)
                for j in range(SJ):
                    for b in range(2):
                        nc.sync.dma_start(KT_sb[:, b, j], k_out[j][b])
                # intensity behind the unpacks: keeps its 4MB off the
                # startup HBM rush; needed only by the normalize passes
                for sj in range(SJ):
                    nc.sync.dma_start(IT_sb[:, sj], IT_d[:, sj])
                for c in range(DC):
                    psl = [mm_ps.tile([P, NT], f32, tag="mm", name="ps")
                           for _ in range(SJ)]
                    for i in range(NDR):
                        for j in range(SJ):
                            nc.tensor.matmul(
                                psl[j][:],
                                WQ_sb[:, 2 * i:2 * i + 2, c * P:(c + 1) * P],
                                X8_sb[:, 2 * i:2 * i + 2,
                                      j * NT:(j + 1) * NT],
                                start=(i == 0), stop=(i == NDR - 1),
                                perf_mode=DR,
                            )
                    for j in range(SJ):
                        nc.vector.tensor_scalar_add(
                            QT_sb[:, c, j * NT:(j + 1) * NT], psl[j][:],
                            BCOL_sb[:, c:c + 1])

                v_half(1)
                # V unpacks last on the SWDGE queue (PV needs them latest)
                for h in range(2):
                    for b in range(2):
                        nc.gpsimd.dma_start(
                            V_sb[:, h, b * DC:(b + 1) * DC, :], v_out[h][b])

            # ---- phase B: scores^T -> exp -> den -> normalize -> +I^T -
            # scores are computed DIRECTLY transposed ([t | s] tiles via
            # lhsT = K^T chunk, rhs = Q^T), which kills the 128 transpose
            # matmuls AND lets each tile depend on only one K gather half:
            # the j0-gather tiles run first so the j1 gather lands in their
            # shadow. Denominators are ones-vector matmuls accumulated in
            # PSUM (sum over the t partition axis), the reciprocal row is
            # broadcast to 128 partitions with a rank-1 f32r outer product,
            # and normalize + intensity-add are two 512-wide DVE ops/tile.
            # j0 tiles of both blocks first, then j1 tiles:
            TT_ORDER = [0, 1, 2, 3, 8, 9, 10, 11, 4, 5, 6, 7, 12, 13, 14, 15]
            with (
                tc.tile_pool(name="rec_pool", bufs=2) as rec_pool,
                tc.tile_pool(name="rbs_pool", bufs=2) as rbs_pool,
            ):
                def sj_scores(sj):
                    for tt in TT_ORDER:
                        blk, j, nl = tt // 8, (tt % 8) // 4, tt % 4
                        ps = mm_ps.tile([P, NT], f32, tag="mm", name="ps")
                        for i in range(NDR):
                            nc.tensor.matmul(
                                ps[:],
                                KT_sb[:, blk, j, 2 * i:2 * i + 2,
                                      nl * P:(nl + 1) * P],
                                QT_sb[:, 2 * i:2 * i + 2,
                                      sj * NT:(sj + 1) * NT],
                                start=(i == 0), stop=(i == NDR - 1),
                                perf_mode=DR,
                            )
                        nc.scalar.activation(
                            AT_sb[:, tt, sj * NT:(sj + 1) * NT], ps[:], Exp,
                            scale=SCALE8)

                def sj_den(sj):
                    dps = den_ps.tile([1, NT], f32, tag="den")
                    for n in range(TC):
                        nc.tensor.matmul(
                            dps[0:1, :], ONESB[:, 0:1],
                            AT_sb[:, n, sj * NT:(sj + 1) * NT],
                            start=(n == 0), stop=(n == TC - 1),
                        )
                    rec = rec_pool.tile([1, NT], f32r, tag="rec")
                    with nc.allow_low_precision(
                            reason="f32r reciprocal row for the rank-1 "
                                   "broadcast; 22-bit mantissa is ample"):
                        nc.vector.reciprocal(rec[0:1, :], dps[0:1, :])
                    rb = rb_ps.tile([P, NT], f32, tag="rb")
                    nc.tensor.matmul(rb[:], ONESR[0:1, :], rec[0:1, :],
                                     start=True, stop=True)
                    RB_sb = rbs_pool.tile([P, NT], bf16, tag="rbs")
                    nc.vector.tensor_copy(out=RB_sb[:], in_=rb[:])
                    return RB_sb

                def sj_normalize(sj, RB_sb):
                    with nc.allow_low_precision(
                            reason="bf16 in-place normalize+intensity add; "
                                   "matches the prior bf16 attn precision"):
                        for tt in range(TC):
                            sl = AT_sb[:, tt, sj * NT:(sj + 1) * NT]
                            nc.vector.tensor_tensor(sl, sl, RB_sb[:], mult)
                            nc.vector.tensor_tensor(
                                sl, sl, IT_sb[:, sj, tt, :], add)

                # den/normalize of sj=0 interleave between the two scores
                # halves: norm0 (DVE) hides under sj=1's matmuls, and PV's
                # first s-half later hides norm1.
                sj_scores(0)
                rb0 = sj_den(0)
                sj_normalize(0, rb0)
                sj_scores(1)
                rb1 = sj_den(1)
                sj_normalize(1, rb1)

            # ---- phase D/E: PV -> out^T, then projection per s-tile ---
            OT_sb = persist.tile([P, DC, SH], bf16, tag="qt_ot")
            with (
                tc.tile_pool(name="wo_pool", bufs=1) as wo_pool,
                tc.tile_pool(name="fin_pool", bufs=3) as fin_pool,
            ):
                WO_sb = wo_pool.tile([P, DC, D], bf16)
                nc.sync.dma_start(WO_sb[:], WO_d[:])
                for sj in range(SJ):
                    for dvi in range(DC):
                        ps = mm_ps.tile([P, NT], f32, tag="mm")
                        for t in range(TC):
                            nc.tensor.matmul(
                                ps[:],
                                V_sb[:, dvi // 4, t,
                                     (dvi % 4) * P:(dvi % 4 + 1) * P],
                                AT_sb[:, t, sj * NT:(sj + 1) * NT],
                                start=(t == 0),
                                stop=False,
                            )
                        # bias: bv (x) (1 + rowsum(I))
                        nc.tensor.matmul(
                            ps[:], BV_sb[0:1, dvi * P:(dvi + 1) * P],
                            RS_sb[0:1, sj * NT:(sj + 1) * NT],
                            start=False, stop=True)
                        nc.vector.tensor_copy(
                            out=OT_sb[:, dvi, sj * NT:(sj + 1) * NT], in_=ps[:])

                    for doi in range(DC):
                        ps = mm_ps.tile([P, NT], f32, tag="mm")
                        for dvc in range(DC):
                            nc.tensor.matmul(
                                ps[:],
                                WO_sb[:, dvc, doi * P:(doi + 1) * P],
                                OT_sb[:, dvc, sj * NT:(sj + 1) * NT],
                                start=(dvc == 0), stop=(dvc == DC - 1),
                            )
                        F_sb = fin_pool.tile([P, NT], f32, tag="fin")
                        nc.vector.tensor_scalar_add(
                            F_sb[:], ps[:],
                            BCOL_sb[:, 2 * DC + doi:2 * DC + doi + 1])
                        nc.sync.dma_start(
                            out_v[:, doi, sj * NT:(sj + 1) * NT], F_sb[:])

    nc.compile()
    return nc


def _get_module():
    if "nc" not in _CACHE:
        _CACHE["nc"] = _build_module()
    return _CACHE["nc"]


def _chunked(a, ncols):
    """[D, ncols] -> [P, DC, ncols] with partition p holding rows c*128+p."""
    return np.ascontiguousarray(
        a.reshape(DC, P, ncols).transpose(1, 0, 2))


def _make_in_maps(inputs):
    X = np.asarray(inputs["X"], dtype=np.float32)
    intensity = np.asarray(inputs["intensity"], dtype=np.float32)
    bf = ml_dtypes.bfloat16
    f8 = ml_dtypes.float8_e4m3
    Wq8 = _chunked(np.ascontiguousarray(
        np.asarray(inputs["Wq"], np.float32).T * 32.0), D).astype(f8)
    Wk8 = _chunked(np.ascontiguousarray(
        np.asarray(inputs["Wk"], np.float32).T * 32.0), D).astype(f8)
    WvT = _chunked(np.ascontiguousarray(
        np.asarray(inputs["Wv"], np.float32).T), D).astype(bf)
    WoT = _chunked(np.ascontiguousarray(
        np.asarray(inputs["Wo"], np.float32).T), D).astype(bf)
    bq, bk, bv, bo = (np.asarray(inputs[k], np.float32).reshape(D)
                      for k in ("bq", "bk", "bv", "bo"))
    BCOL = np.concatenate(
        [b.reshape(DC, P).T for b in (32.0 * bq, 32.0 * bk, bo)], axis=1
    ).astype(np.float32)  # [128, 24]

    in_maps = []
    for c in range(8):
        b, h = c // 2, c % 2
        XT = np.ascontiguousarray(X[b, h * SH:(h + 1) * SH, :].T)
        XQT = _chunked(XT, SH).astype(bf)
        X8T = _chunked(XT, SH).astype(f8)
        Islc = intensity[b, h * SH:(h + 1) * SH, :]
        # I^T chunked sj-major: IT[p, sj, tt, n] = I^T[tt*128+p, sj*512+n],
        # matching the directly-transposed attn tiles; each sj-half loads
        # as one contiguous 16KB/partition DMA
        IT = np.ascontiguousarray(
            Islc.T.reshape(TC, P, SJ, NT).transpose(1, 2, 0, 3)).astype(bf)
        rows = 1.0 + Islc.sum(axis=1, dtype=np.float64).astype(np.float32)
        BROW = np.concatenate([bv, rows]).reshape(1, D + SH)
        in_maps.append({
            "X8T": X8T, "XQT": XQT, "WQ8": Wq8, "WK8": Wk8,
            "WVT": WvT, "WOT": WoT,
            "BCOL": BCOL, "BROW": BROW, "IT": IT,
        })
    return in_maps


def _gather(results):
    out = np.empty((4, S, D), dtype=np.float32)
    for c in range(8):
        b, h = c // 2, c % 2
        out[b, h * SH:(h + 1) * SH, :] = results[c]["OUTT"].T
    return out


def kernel(**inputs):
    from concourse import bass_utils

    in_maps = _make_in_maps(inputs)
    nc = _get_module()
    res = bass_utils.run_bass_kernel_spmd(nc, in_maps, core_ids=list(range(8)))
    return _gather(res.results)


# revision 95
# speedup vs baseline: 1.1478x; 1.0239x over previous
"""Trainium2 Bass kernel for nn_Attention_54254026883778.

Single-head attention with an additive post-softmax intensity term:
    q/k/v = X @ W{q,k,v}.T + b;  scores = q k^T / sqrt(D)
    attn  = softmax(scores) + intensity;  out = (attn @ v) @ Wo.T + bo

Sharding: 8 cores = 4 batches x 2 sequence halves. Each core computes
K^T and V for its own 1024 rows; the partner half arrives via 2-rank
AllGathers through shared DRAM (global t-order, SPMD-static).

Precision plan: Q/K projections and the scores matmul run in fp8e4m3
with DoubleRow (2x PE throughput). Host ships X and 32*W{q,k}^T in fp8
(the x32 shift keeps the tiny weights in fp8's normal range); the exp
activation folds the 1/(32*32) back in on top of 1/sqrt(D). Softmax
errors are ~1000x attenuated in the output (softmax rows sum to 1 while
the additive intensity rows sum to ~1024), so fp8 there is safe. The
V projection, PV and output projection stay bf16.

Dataflow per core (own 1024 query rows, full 2048 keys):
    warmup   dummy matmuls under the initial DMA so the PE HAM
             clock-gate opens before real work arrives
    K^T      [dout | t-own]  fp8 DR, j-outer; per-j fp8 AllGather
    V        [t-own | dv]    bf16, dv-half-outer; per-half AllGather
    Q^T      [dout | s]      fp8 DR
    scores   [s | t]         fp8 DR -> exp on ACT with fused
        row-accumulate -> 1/den on DVE -> diag(recip) ->
        attn^T tile = E-slice.T @ diag(recip)  (PE transposes+normalizes)
        -> DVE adds intensity^T while draining PSUM -> attn^T [t | s]
    out^T    [dv | s]    = V-chunk.T @ attn^T   (bf16)
    final^T  [do | s]    = WoT-chunk.T @ out^T  -> DRAM, host transposes
Biases: q/k (x32) and o enter as per-partition adds during PSUM extract;
v enters as a rank-1 fp32r matmul bv (x) (1 + rowsum(I)).

DMA plan (the previous round stalled 50us on queue head-blocking):
  - big inputs are host-swizzled to [P, chunks, cols] so each is ONE
    trigger with 8-16KB contiguous per-partition packets
  - all collective staging/unpacks ride the SWDGE (gpsimd) queue in an
    emission order that never parks an unpack in front of a stage
  - intensity loads + WO + outputs stay on the sync HWDGE queue
"""

import numpy as np
import ml_dtypes

P = 128
D = 1024
S = 2048          # keys per batch (full sequence)
SH = 1024         # query rows owned by each core
DC = D // P       # 8  contraction chunks over model dim
TC = S // P       # 16 t (key) chunks
NT = 512          # matmul moving free dim / psum bank
SJ = SH // NT     # 2  s-tiles of own rows
TJ = S // NT      # 4  t-tiles
SCALE = 1.0 / 32.0        # 1/sqrt(D)
SCALE8 = SCALE / 1024.0   # undo the x32 on q and k

_CACHE = {}


def _build_module():
    import concourse.bass as bass
    import concourse.tile as tile
    import concourse.mybir as mybir
    from concourse import bacc
    from concourse.masks import make_identity

    f32 = mybir.dt.float32
    f32r = mybir.dt.float32r
    bf16 = mybir.dt.bfloat16
    fp8 = mybir.dt.float8e4
    DR = mybir.MatmulPerfMode.DoubleRow
    Exp = mybir.ActivationFunctionType.Exp
    add = mybir.AluOpType.add
    mult = mybir.AluOpType.mult

    nc = bacc.Bacc("TRN2", target_bir_lowering=False, debug=False,
                   num_devices=8)

    X8_d = nc.dram_tensor("X8T", [P, DC, SH], fp8, kind="ExternalInput")
    XQ_d = nc.dram_tensor("XQT", [P, DC, SH], bf16, kind="ExternalInput")
    WQ_d = nc.dram_tensor("WQ8", [P, DC, D], fp8, kind="ExternalInput")
    WK_d = nc.dram_tensor("WK8", [P, DC, D], fp8, kind="ExternalInput")
    WV_d = nc.dram_tensor("WVT", [P, DC, D], bf16, kind="ExternalInput")
    WO_d = nc.dram_tensor("WOT", [P, DC, D], bf16, kind="ExternalInput")
    BCOL_d = nc.dram_tensor("BCOL", [P, 3 * DC], f32, kind="ExternalInput")
    BROW_d = nc.dram_tensor("BROW", [1, D + SH], f32, kind="ExternalInput")
    IT_d = nc.dram_tensor("IT", [P, SJ, TC, NT], bf16, kind="ExternalInput")
    OUT_d = nc.dram_tensor("OUTT", [D, SH], f32, kind="ExternalOutput")

    out_v = OUT_d[:].rearrange("(c p) s -> p c s", p=P)

    GROUPS = [[0, 1], [2, 3], [4, 5], [6, 7]]
    NDR = DC // 2  # 4 contraction pair-chunks for DoubleRow

    with tile.TileContext(nc) as tc:
        with (
            tc.tile_pool(name="persist", bufs=1) as persist,
            tc.tile_pool(name="mm_ps", bufs=5, space="PSUM") as mm_ps,
            tc.tile_pool(name="den_ps", bufs=2, space="PSUM") as den_ps,
            tc.tile_pool(name="rb_ps", bufs=1, space="PSUM") as rb_ps,
            tc.tile_pool(name="dram", bufs=1, space="DRAM") as dram_pool,
        ):
            # ---- persistent tiles -------------------------------------
            KT_sb = persist.tile([P, 2, SJ, DC, NT], fp8)   # K^T gathered
            V_sb = persist.tile([P, 2, TC, NT], bf16)       # V [t | half,dv]
            QT_sb = persist.tile([P, DC, SH], fp8, tag="qt_ot")
            X8_sb = persist.tile([P, DC, SH], fp8)          # X^T own, fp8
            XQ_sb = persist.tile([P, DC, SH], bf16, tag="xq_at")
            KL_sb = persist.tile([P, SJ, DC, NT], fp8)      # K^T local (x32)
            VL_sb = persist.tile([P, SJ, DC, NT], bf16)     # local V halves
            AT_sb = persist.tile([P, TC, SH], bf16, tag="xq_at")  # attn^T
            IT_sb = persist.tile([P, SJ, TC, NT], bf16)     # intensity^T

            ident = persist.tile([P, P], bf16)
            make_identity(nc, ident)
            WARM_sb = persist.tile([P, NT], bf16)
            nc.vector.memset(WARM_sb[:], 0.0)
            # ones column (bf16) for the den row-matmuls, ones row (f32r)
            # for the recip broadcast outer product
            ONESB = persist.tile([P, 1], bf16)
            nc.vector.memset(ONESB[:], 1.0)
            ONESR = persist.tile([1, P], f32r)
            with tc.tile_pool(name="ones_pool", bufs=1) as ones_pool:
                ones_f = ones_pool.tile([1, P], f32)
                nc.vector.memset(ones_f[:], 1.0)
                nc.vector.tensor_copy(out=ONESR[:], in_=ones_f[:])
            # bq*32|bk*32|bo as per-partition columns, added on PSUM extract
            BCOL_sb = persist.tile([P, 3 * DC], f32)
            nc.sync.dma_start(BCOL_sb[:], BCOL_d[:])
            # bv and the attn rowsums feed the PV rank-1 bias matmul; fp32r
            # operands must come from a rounding instruction, so stage the
            # DMA through a DVE copy.
            BROW_r = persist.tile([1, D + SH], f32r)
            with tc.tile_pool(name="brow_pool", bufs=1) as brow_pool:
                BROW_ld = brow_pool.tile([1, D + SH], f32)
                nc.sync.dma_start(BROW_ld[:], BROW_d[:])
                nc.vector.tensor_copy(out=BROW_r[:], in_=BROW_ld[:])
            BV_sb = BROW_r[0:1, 0:D]
            RS_sb = BROW_r[0:1, D:D + SH]                   # 1 + rowsum(I)

            # ---- PE warmup: dummy matmuls under the initial DMA -------
            for _ in range(20):
                wps = mm_ps.tile([P, NT], f32, tag="mm", name="warm")
                nc.tensor.matmul(wps[:], ident[:], WARM_sb[:],
                                 start=True, stop=True)


            # collective DRAM tiles: K per j-half (fp8), V per dv-half
            k_in = [dram_pool.tile([P, DC, NT], fp8, name=f"k_in{j}")
                    for j in range(SJ)]
            k_out = [dram_pool.tile([2, P, DC, NT], fp8, name=f"k_out{j}")
                     for j in range(SJ)]
            v_in = [dram_pool.tile([P, DC, NT], bf16, name=f"v_in{h}")
                    for h in range(2)]
            v_out = [dram_pool.tile([2, P, DC, NT], bf16, name=f"v_out{h}")
                     for h in range(2)]

            # ---- phase A: K full-local, V-local/gather, Q -------------
            with (
                tc.tile_pool(name="w8pool", bufs=1) as w8pool,
                tc.tile_pool(name="wvpool", bufs=1) as wvpool,
            ):
                # All inputs ride the sync queue in deadline order (the
                # proven config); WQ8 shares the single w8pool buffer with
                # WK8 so its load is WAR-gated until K consumed the weights.
                WK_sb = w8pool.tile([P, DC, D], fp8, tag="w8")
                nc.sync.dma_start(X8_sb[:], X8_d[:])
                nc.sync.dma_start(WK_sb[:], WK_d[:])
                WV_sb = wvpool.tile([P, DC, D], bf16)
                nc.sync.dma_start(XQ_sb[:], XQ_d[:])
                nc.sync.dma_start(WV_sb[:], WV_d[:])

                # K^T local [dout, t-own] fp8 DoubleRow, j-outer; stage and
                # gather each j-half on the SWDGE as soon as it is done so
                # both doorbells ring by ~36us and the wire work finishes
                # well before the Q projection does.
                for j in range(SJ):
                    for c in range(DC):
                        ps = mm_ps.tile([P, NT], f32, tag="mm", name="ps")
                        for i in range(NDR):
                            nc.tensor.matmul(
                                ps[:],
                                WK_sb[:, 2 * i:2 * i + 2, c * P:(c + 1) * P],
                                X8_sb[:, 2 * i:2 * i + 2,
                                      j * NT:(j + 1) * NT],
                                start=(i == 0), stop=(i == NDR - 1),
                                perf_mode=DR,
                            )
                        nc.vector.tensor_scalar_add(
                            KL_sb[:, j, c, :], ps[:],
                            BCOL_sb[:, DC + c:DC + c + 1])
                    nc.gpsimd.dma_start(k_in[j][:], KL_sb[:, j])
                    nc.gpsimd.collective_compute(
                        "AllGather", mybir.AluOpType.bypass,
                        replica_groups=GROUPS,
                        ins=[k_in[j].opt()], outs=[k_out[j].opt()])

                # V local, dv-half-outer (h = output half); VL is [P,h,t,dv]
                # so each half stages as one contiguous 8KB/partition DMA.
                # The SECOND half runs AFTER the Q projection: it fills the
                # PE while the K AllGather results land, so the scores never
                # wait on the collective chain (and either order of {V-h1,
                # Q} keeps that property if the scheduler swaps them).
                def v_half(h):
                    for t in range(DC):
                        ps = mm_ps.tile([P, NT], f32, tag="mm", name="ps")
                        for dc in range(DC):
                            nc.tensor.matmul(
                                ps[:],
                                XQ_sb[:, dc, t * P:(t + 1) * P],
                                WV_sb[:, dc, h * NT:(h + 1) * NT],
                                start=(dc == 0),
                                stop=(dc == DC - 1),
                            )
                        nc.vector.tensor_copy(
                            out=VL_sb[:, h, t, :], in_=ps[:])
                    nc.gpsimd.dma_start(v_in[h][:], VL_sb[:, h])
                    nc.gpsimd.collective_compute(
                        "AllGather", mybir.AluOpType.bypass,
                        replica_groups=GROUPS,
                        ins=[v_in[h].opt()], outs=[v_out[h].opt()])

                v_half(0)

                # Q^T [dout, s-own] fp8 DoubleRow; WQ8 reuses WK8's buffer
                # (bufs=1 WAR-gates its DMA behind the last K matmul, which
                # keeps it out of the startup HBM rush but lands ~40us, well
                # before Q starts).
                WQ_sb = w8pool.tile([P, DC, D], fp8, tag="w8")
                nc.sync.dma_start(WQ_sb[:], WQ_d[:])
                # K unpacks on the sync HWDGE queue, emitted after every
                # input trigger so their ccK semaphore waits can only
                # head-block the intensity loads (needed ~25us later)
                for j in range(SJ):
                    for b in range(2):
                        nc.sync.dma_start(KT_sb[:, b, j], k_out[j][b])
                # intensity behind the unpacks: keeps its 4MB off the
                # startup HBM rush; needed only by the normalize passes
                for sj in range(SJ):
                    nc.sync.dma_start(IT_sb[:, sj], IT_d[:, sj])
                for c in range(DC):
                    psl = [mm_ps.tile([P, NT], f32, tag="mm", name="ps")
                           for _ in range(SJ)]
                    for i in range(NDR):
                        for j in range(SJ):
                            nc.tensor.matmul(
                                psl[j][:],
                                WQ_sb[:, 2 * i:2 * i + 2, c * P:(c + 1) * P],
                                X8_sb[:, 2 * i:2 * i + 2,
                                      j * NT:(j + 1) * NT],
                                start=(i == 0), stop=(i == NDR - 1),
                                perf_mode=DR,
                            )
                    for j in range(SJ):
                        nc.vector.tensor_scalar_add(
                            QT_sb[:, c, j * NT:(j + 1) * NT], psl[j][:],
                            BCOL_sb[:, c:c + 1])

                v_half(1)
                # V unpacks last on the SWDGE queue (PV needs them latest)
                for h in range(2):
                    for b in range(2):
                        nc.gpsimd.dma_start(
                            V_sb[:, h, b * DC:(b + 1) * DC, :], v_out[h][b])

            # ---- phase B: scores^T -> exp -> den -> normalize -> +I^T -
            # scores are computed DIRECTLY transposed ([t | s] tiles via
            # lhsT = K^T chunk, rhs = Q^T), which kills the 128 transpose
            # matmuls AND lets each tile depend on only one K gather half:
            # the j0-gather tiles run first so the j1 gather lands in their
            # shadow. Denominators are ones-vector matmuls accumulated in
            # PSUM (sum over the t partition axis), the reciprocal row is
            # broadcast to 128 partitions with a rank-1 f32r outer product,
            # and normalize + intensity-add are two 512-wide DVE ops/tile.
            # j0 tiles of both blocks first, then j1 tiles:
            TT_ORDER = [0, 1, 2, 3, 8, 9, 10, 11, 4, 5, 6, 7, 12, 13, 14, 15]
            with (
                tc.tile_pool(name="rec_pool", bufs=2) as rec_pool,
                tc.tile_pool(name="rbs_pool", bufs=2) as rbs_pool,
            ):
                def sj_scores(sj):
                    for tt in TT_ORDER:
                        blk, j, nl = tt // 8, (tt % 8) // 4, tt % 4
                        ps = mm_ps.tile([P, NT], f32, tag="mm", name="ps")
                        for i in range(NDR):
                            nc.tensor.matmul(
                                ps[:],
                                KT_sb[:, blk, j, 2 * i:2 * i + 2,
                                      nl * P:(nl + 1) * P],
                                QT_sb[:, 2 * i:2 * i + 2,
                                      sj * NT:(sj + 1) * NT],
                                start=(i == 0), stop=(i == NDR - 1),
                                perf_mode=DR,
                            )
                        nc.scalar.activation(
                            AT_sb[:, tt, sj * NT:(sj + 1) * NT], ps[:], Exp,
                            scale=SCALE8)

                def sj_den(sj):
                    dps = den_ps.tile([1, NT], f32, tag="den")
                    for n in range(TC):
                        nc.tensor.matmul(
                            dps[0:1, :], ONESB[:, 0:1],
                            AT_sb[:, n, sj * NT:(sj + 1) * NT],
                            start=(n == 0), stop=(n == TC - 1),
                        )
                    rec = rec_pool.tile([1, NT], f32r, tag="rec")
                    with nc.allow_low_precision(
                            reason="f32r reciprocal row for the rank-1 "
                                   "broadcast; 22-bit mantissa is ample"):
                        nc.vector.reciprocal(rec[0:1, :], dps[0:1, :])
                    rb = rb_ps.tile([P, NT], f32, tag="rb")
                    nc.tensor.matmul(rb[:], ONESR[0:1, :], rec[0:1, :],
                                     start=True, stop=True)
                    RB_sb = rbs_pool.tile([P, NT], bf16, tag="rbs")
                    nc.vector.tensor_copy(out=RB_sb[:], in_=rb[:])
                    return RB_sb

                def sj_normalize(sj, RB_sb):
                    # mult on DVE, intensity-add on the otherwise-idle
                    # GpSimd (SBUF-only operands): halves the DVE queue
                    # occupancy so the next half's reciprocal/broadcast
                    # unblocks sooner, and halves the normalize latency
                    # that PV's first tiles chase.
                    with nc.allow_low_precision(
                            reason="bf16 in-place normalize+intensity add; "
                                   "matches the prior bf16 attn precision"):
                        for tt in range(TC):
                            sl = AT_sb[:, tt, sj * NT:(sj + 1) * NT]
                            nc.vector.tensor_tensor(sl, sl, RB_sb[:], mult)
                            nc.gpsimd.tensor_tensor(
                                sl, sl, IT_sb[:, sj, tt, :], add)

                # den/normalize of sj=0 interleave between the two scores
                # halves: norm0 (DVE) hides under sj=1's matmuls, and PV's
                # first s-half later hides norm1.
                sj_scores(0)
                rb0 = sj_den(0)
                sj_normalize(0, rb0)
                sj_scores(1)
                rb1 = sj_den(1)
                sj_normalize(1, rb1)

            # ---- phase D/E: PV -> out^T, then projection per s-tile ---
            OT_sb = persist.tile([P, DC, SH], bf16, tag="qt_ot")
            with (
                tc.tile_pool(name="wo_pool", bufs=1) as wo_pool,
                tc.tile_pool(name="fin_pool", bufs=3) as fin_pool,
            ):
                WO_sb = wo_pool.tile([P, DC, D], bf16)
                nc.sync.dma_start(WO_sb[:], WO_d[:])
                for sj in range(SJ):
                    for dvi in range(DC):
                        ps = mm_ps.tile([P, NT], f32, tag="mm")
                        for t in range(TC):
                            nc.tensor.matmul(
                                ps[:],
                                V_sb[:, dvi // 4, t,
                                     (dvi % 4) * P:(dvi % 4 + 1) * P],
                                AT_sb[:, t, sj * NT:(sj + 1) * NT],
                                start=(t == 0),
                                stop=False,
                            )
                        # bias: bv (x) (1 + rowsum(I))
                        nc.tensor.matmul(
                            ps[:], BV_sb[0:1, dvi * P:(dvi + 1) * P],
                            RS_sb[0:1, sj * NT:(sj + 1) * NT],
                            start=False, stop=True)
                        nc.vector.tensor_copy(
                            out=OT_sb[:, dvi, sj * NT:(sj + 1) * NT], in_=ps[:])

                    for doi in range(DC):
                        ps = mm_ps.tile([P, NT], f32, tag="mm")
                        for dvc in range(DC):
                            nc.tensor.matmul(
                                ps[:],
                                WO_sb[:, dvc, doi * P:(doi + 1) * P],
                                OT_sb[:, dvc, sj * NT:(sj + 1) * NT],
                                start=(dvc == 0), stop=(dvc == DC - 1),
                            )
                        F_sb = fin_pool.tile([P, NT], f32, tag="fin")
                        nc.vector.tensor_scalar_add(
                            F_sb[:], ps[:],
                            BCOL_sb[:, 2 * DC + doi:2 * DC + doi + 1])
                        nc.sync.dma_start(
                            out_v[:, doi, sj * NT:(sj + 1) * NT], F_sb[:])

    nc.compile()
    return nc


def _get_module():
    if "nc" not in _CACHE:
        _CACHE["nc"] = _build_module()
    return _CACHE["nc"]


def _chunked(a, ncols):
    """[D, ncols] -> [P, DC, ncols] with partition p holding rows c*128+p."""
    return np.ascontiguousarray(
        a.reshape(DC, P, ncols).transpose(1, 0, 2))


def _make_in_maps(inputs):
    X = np.asarray(inputs["X"], dtype=np.float32)
    intensity = np.asarray(inputs["intensity"], dtype=np.float32)
    bf = ml_dtypes.bfloat16
    f8 = ml_dtypes.float8_e4m3
    Wq8 = _chunked(np.ascontiguousarray(
        np.asarray(inputs["Wq"], np.float32).T * 32.0), D).astype(f8)
    Wk8 = _chunked(np.ascontiguousarray(
        np.asarray(inputs["Wk"], np.float32).T * 32.0), D).astype(f8)
    WvT = _chunked(np.ascontiguousarray(
        np.asarray(inputs["Wv"], np.float32).T), D).astype(bf)
    WoT = _chunked(np.ascontiguousarray(
        np.asarray(inputs["Wo"], np.float32).T), D).astype(bf)
    bq, bk, bv, bo = (np.asarray(inputs[k], np.float32).reshape(D)
                      for k in ("bq", "bk", "bv", "bo"))
    BCOL = np.concatenate(
        [b.reshape(DC, P).T for b in (32.0 * bq, 32.0 * bk, bo)], axis=1
    ).astype(np.float32)  # [128, 24]

    in_maps = []
    for c in range(8):
        b, h = c // 2, c % 2
        XT = np.ascontiguousarray(X[b, h * SH:(h + 1) * SH, :].T)
        XQT = _chunked(XT, SH).astype(bf)
        X8T = _chunked(XT, SH).astype(f8)
        Islc = intensity[b, h * SH:(h + 1) * SH, :]
        # I^T chunked sj-major: IT[p, sj, tt, n] = I^T[tt*128+p, sj*512+n],
        # matching the directly-transposed attn tiles; each sj-half loads
        # as one contiguous 16KB/partition DMA
        IT = np.ascontiguousarray(
            Islc.T.reshape(TC, P, SJ, NT).transpose(1, 2, 0, 3)).astype(bf)
        rows = 1.0 + Islc.sum(axis=1, dtype=np.float64).astype(np.float32)
        BROW = np.concatenate([bv, rows]).reshape(1, D + SH)
        in_maps.append({
            "X8T": X8T, "XQT": XQT, "WQ8": Wq8, "WK8": Wk8,
            "WVT": WvT, "WOT": WoT,
            "BCOL": BCOL, "BROW": BROW, "IT": IT,
        })
    return in_maps


def _gather(results):
    out = np.empty((4, S, D), dtype=np.float32)
    for c in range(8):
        b, h = c // 2, c % 2
        out[b, h * SH:(h + 1) * SH, :] = results[c]["OUTT"].T
    return out


def kernel(**inputs):
    from concourse import bass_utils

    in_maps = _make_in_maps(inputs)
    nc = _get_module()
    res = bass_utils.run_bass_kernel_spmd(nc, in_maps, core_ids=list(range(8)))
    return _gather(res.results)


# revision 97
# speedup vs baseline: 1.1899x; 1.0366x over previous
"""Trainium2 Bass kernel for nn_Attention_54254026883778.

Single-head attention with an additive post-softmax intensity term:
    q/k/v = X @ W{q,k,v}.T + b;  scores = q k^T / sqrt(D)
    attn  = softmax(scores) + intensity;  out = (attn @ v) @ Wo.T + bo

Sharding: 8 cores = 4 batches x 2 sequence halves. Each core computes
K^T and V for its own 1024 rows; the partner half arrives via 2-rank
AllGathers through shared DRAM (global t-order, SPMD-static).

Precision plan: Q/K projections and the scores matmul run in fp8e4m3
with DoubleRow (2x PE throughput). Host ships X and 32*W{q,k}^T in fp8
(the x32 shift keeps the tiny weights in fp8's normal range); the exp
activation folds the 1/(32*32) back in on top of 1/sqrt(D). Softmax
errors are ~1000x attenuated in the output (softmax rows sum to 1 while
the additive intensity rows sum to ~1024), so fp8 there is safe. The
V projection, PV and output projection stay bf16.

Dataflow per core (own 1024 query rows, full 2048 keys):
    warmup   dummy matmuls under the initial DMA so the PE HAM
             clock-gate opens before real work arrives
    K^T      [dout | t-own]  fp8 DR, j-outer; per-j fp8 AllGather
    V        [t-own | dv]    bf16, dv-half-outer; per-half AllGather
    Q^T      [dout | s]      fp8 DR
    scores   [s | t]         fp8 DR -> exp on ACT with fused
        row-accumulate -> 1/den on DVE -> diag(recip) ->
        attn^T tile = E-slice.T @ diag(recip)  (PE transposes+normalizes)
        -> DVE adds intensity^T while draining PSUM -> attn^T [t | s]
    out^T    [dv | s]    = V-chunk.T @ attn^T   (bf16)
    final^T  [do | s]    = WoT-chunk.T @ out^T  -> DRAM, host transposes
Biases: q/k (x32) and o enter as per-partition adds during PSUM extract;
v enters as a rank-1 fp32r matmul bv (x) (1 + rowsum(I)).

DMA plan (the previous round stalled 50us on queue head-blocking):
  - big inputs are host-swizzled to [P, chunks, cols] so each is ONE
    trigger with 8-16KB contiguous per-partition packets
  - all collective staging/unpacks ride the SWDGE (gpsimd) queue in an
    emission order that never parks an unpack in front of a stage
  - intensity loads + WO + outputs stay on the sync HWDGE queue
"""

import numpy as np
import ml_dtypes

P = 128
D = 1024
S = 2048          # keys per batch (full sequence)
SH = 1024         # query rows owned by each core
DC = D // P       # 8  contraction chunks over model dim
TC = S // P       # 16 t (key) chunks
NT = 512          # matmul moving free dim / psum bank
SJ = SH // NT     # 2  s-tiles of own rows
TJ = S // NT      # 4  t-tiles
SCALE = 1.0 / 32.0        # 1/sqrt(D)
SCALE8 = SCALE / 1024.0   # undo the x32 on q and k

_CACHE = {}


def _build_module():
    import concourse.bass as bass
    import concourse.tile as tile
    import concourse.mybir as mybir
    from concourse import bacc
    from concourse.masks import make_identity

    f32 = mybir.dt.float32
    f32r = mybir.dt.float32r
    bf16 = mybir.dt.bfloat16
    fp8 = mybir.dt.float8e4
    DR = mybir.MatmulPerfMode.DoubleRow
    Exp = mybir.ActivationFunctionType.Exp
    add = mybir.AluOpType.add

    nc = bacc.Bacc("TRN2", target_bir_lowering=False, debug=False,
                   num_devices=8)

    X8_d = nc.dram_tensor("X8T", [P, DC, SH], fp8, kind="ExternalInput")
    XQ_d = nc.dram_tensor("XQT", [P, DC, SH], bf16, kind="ExternalInput")
    WQ_d = nc.dram_tensor("WQ8", [P, DC, D], fp8, kind="ExternalInput")
    WK_d = nc.dram_tensor("WK8", [P, DC, D], fp8, kind="ExternalInput")
    WV_d = nc.dram_tensor("WVT", [P, DC, D], bf16, kind="ExternalInput")
    WO_d = nc.dram_tensor("WOT", [P, DC, D], bf16, kind="ExternalInput")
    BCOL_d = nc.dram_tensor("BCOL", [P, 3 * DC], f32, kind="ExternalInput")
    BROW_d = nc.dram_tensor("BROW", [1, D + SH], f32, kind="ExternalInput")
    IT_d = nc.dram_tensor("IT", [SH, S], bf16, kind="ExternalInput")
    OUT_d = nc.dram_tensor("OUTT", [D, SH], f32, kind="ExternalOutput")

    out_v = OUT_d[:].rearrange("(c p) s -> p c s", p=P)

    GROUPS = [[0, 1], [2, 3], [4, 5], [6, 7]]
    NDR = DC // 2  # 4 contraction pair-chunks for DoubleRow

    with tile.TileContext(nc) as tc:
        with (
            tc.tile_pool(name="persist", bufs=1) as persist,
            tc.tile_pool(name="mm_ps", bufs=6, space="PSUM") as mm_ps,
            tc.tile_pool(name="tr_ps", bufs=2, space="PSUM") as tr_ps,
            tc.tile_pool(name="dram", bufs=1, space="DRAM") as dram_pool,
        ):
            # ---- persistent tiles -------------------------------------
            KT_sb = persist.tile([P, 2, SJ, DC, NT], fp8)   # K^T gathered
            V_sb = persist.tile([P, 2, TC, NT], bf16)       # V [t | half,dv]
            QT_sb = persist.tile([P, DC, SH], fp8, tag="qt_ot")
            X8_sb = persist.tile([P, DC, SH], fp8)          # X^T own, fp8
            XQ_sb = persist.tile([P, DC, SH], bf16, tag="xq_at")
            KL_sb = persist.tile([P, SJ, DC, NT], fp8)      # K^T local (x32)
            VL_sb = persist.tile([P, SJ, DC, NT], bf16)     # local V halves
            AT_sb = persist.tile([P, TC, SH], bf16, tag="xq_at")  # attn^T
            ACC_sb = persist.tile([P, DC, TJ], f32)
            IT_sb = persist.tile([P, DC, S], bf16)          # intensity^T

            ident = persist.tile([P, P], bf16)
            make_identity(nc, ident)
            WARM_sb = persist.tile([P, NT], bf16)
            nc.vector.memset(WARM_sb[:], 0.0)
            # bq*32|bk*32|bo as per-partition columns, added on PSUM extract
            BCOL_sb = persist.tile([P, 3 * DC], f32)
            nc.sync.dma_start(BCOL_sb[:], BCOL_d[:])
            # bv and the attn rowsums feed the PV rank-1 bias matmul; fp32r
            # operands must come from a rounding instruction, so stage the
            # DMA through a DVE copy.
            BROW_r = persist.tile([1, D + SH], f32r)
            with tc.tile_pool(name="brow_pool", bufs=1) as brow_pool:
                BROW_ld = brow_pool.tile([1, D + SH], f32)
                nc.sync.dma_start(BROW_ld[:], BROW_d[:])
                nc.vector.tensor_copy(out=BROW_r[:], in_=BROW_ld[:])
            BV_sb = BROW_r[0:1, 0:D]
            RS_sb = BROW_r[0:1, D:D + SH]                   # 1 + rowsum(I)

            # ---- PE warmup: dummy matmuls under the initial DMA -------
            for _ in range(20):
                wps = mm_ps.tile([P, NT], f32, tag="mm", name="warm")
                nc.tensor.matmul(wps[:], ident[:], WARM_sb[:],
                                 start=True, stop=True)


            # collective DRAM tiles: K per j-half (fp8), V per dv-half
            k_in = [dram_pool.tile([P, DC, NT], fp8, name=f"k_in{j}")
                    for j in range(SJ)]
            k_out = [dram_pool.tile([2, P, DC, NT], fp8, name=f"k_out{j}")
                     for j in range(SJ)]
            v_in = [dram_pool.tile([P, DC, NT], bf16, name=f"v_in{h}")
                    for h in range(2)]
            v_out = [dram_pool.tile([2, P, DC, NT], bf16, name=f"v_out{h}")
                     for h in range(2)]

            # ---- phase A: K full-local, V-local/gather, Q -------------
            with (
                tc.tile_pool(name="w8pool", bufs=1) as w8pool,
                tc.tile_pool(name="wvpool", bufs=1) as wvpool,
            ):
                # All inputs ride the sync queue in deadline order (the
                # proven config); WQ8 shares the single w8pool buffer with
                # WK8 so its load is WAR-gated until K consumed the weights.
                WK_sb = w8pool.tile([P, DC, D], fp8, tag="w8")
                nc.sync.dma_start(X8_sb[:], X8_d[:])
                nc.sync.dma_start(WK_sb[:], WK_d[:])
                WV_sb = wvpool.tile([P, DC, D], bf16)
                nc.sync.dma_start(XQ_sb[:], XQ_d[:])
                nc.sync.dma_start(WV_sb[:], WV_d[:])
                # intensity prefetch on the otherwise-empty SCALAR queue:
                # keeps its 4MB off the sync queue, so the K unpacks (the
                # final leg of the collective wall, ~85us) and the WQ8 load
                # are not slowed by it. Any HBM it steals at t=0 only
                # delays pre-wall compute, which the wall absorbs anyway.
                nc.scalar.dma_start(
                    IT_sb[:], IT_d[:].rearrange("(si p) f -> p si f", p=P))

                # K^T local [dout, t-own] fp8 DoubleRow, j-outer; stage and
                # gather each j-half on the SWDGE as soon as it is done so
                # both doorbells ring by ~36us and the wire work finishes
                # well before the Q projection does.
                for j in range(SJ):
                    for c in range(DC):
                        ps = mm_ps.tile([P, NT], f32, tag="mm", name="ps")
                        for i in range(NDR):
                            nc.tensor.matmul(
                                ps[:],
                                WK_sb[:, 2 * i:2 * i + 2, c * P:(c + 1) * P],
                                X8_sb[:, 2 * i:2 * i + 2,
                                      j * NT:(j + 1) * NT],
                                start=(i == 0), stop=(i == NDR - 1),
                                perf_mode=DR,
                            )
                        nc.vector.tensor_scalar_add(
                            KL_sb[:, j, c, :], ps[:],
                            BCOL_sb[:, DC + c:DC + c + 1])
                    nc.gpsimd.dma_start(k_in[j][:], KL_sb[:, j])
                    nc.gpsimd.collective_compute(
                        "AllGather", mybir.AluOpType.bypass,
                        replica_groups=GROUPS,
                        ins=[k_in[j].opt()], outs=[k_out[j].opt()])

                # V local, dv-half-outer (h = output half); VL is [P,h,t,dv]
                # so each half stages as one contiguous 8KB/partition DMA.
                # The SECOND half runs AFTER the Q projection: it fills the
                # PE while the K AllGather results land, so the scores never
                # wait on the collective chain (and either order of {V-h1,
                # Q} keeps that property if the scheduler swaps them).
                def v_half(h):
                    for t in range(DC):
                        ps = mm_ps.tile([P, NT], f32, tag="mm", name="ps")
                        for dc in range(DC):
                            nc.tensor.matmul(
                                ps[:],
                                XQ_sb[:, dc, t * P:(t + 1) * P],
                                WV_sb[:, dc, h * NT:(h + 1) * NT],
                                start=(dc == 0),
                                stop=(dc == DC - 1),
                            )
                        nc.vector.tensor_copy(
                            out=VL_sb[:, h, t, :], in_=ps[:])
                    nc.gpsimd.dma_start(v_in[h][:], VL_sb[:, h])
                    nc.gpsimd.collective_compute(
                        "AllGather", mybir.AluOpType.bypass,
                        replica_groups=GROUPS,
                        ins=[v_in[h].opt()], outs=[v_out[h].opt()])

                v_half(0)

                # Q^T [dout, s-own] fp8 DoubleRow; WQ8 reuses WK8's buffer
                # (bufs=1 WAR-gates its DMA behind the last K matmul)
                WQ_sb = w8pool.tile([P, DC, D], fp8, tag="w8")
                nc.sync.dma_start(WQ_sb[:], WQ_d[:])
                # K unpacks on the sync HWDGE queue, emitted after every
                # input trigger so their ccK semaphore waits can only
                # head-block the phase-D WO/output triggers (far later)
                for j in range(SJ):
                    for b in range(2):
                        nc.sync.dma_start(KT_sb[:, b, j], k_out[j][b])
                for c in range(DC):
                    psl = [mm_ps.tile([P, NT], f32, tag="mm", name="ps")
                           for _ in range(SJ)]
                    for i in range(NDR):
                        for j in range(SJ):
                            nc.tensor.matmul(
                                psl[j][:],
                                WQ_sb[:, 2 * i:2 * i + 2, c * P:(c + 1) * P],
                                X8_sb[:, 2 * i:2 * i + 2,
                                      j * NT:(j + 1) * NT],
                                start=(i == 0), stop=(i == NDR - 1),
                                perf_mode=DR,
                            )
                    for j in range(SJ):
                        nc.vector.tensor_scalar_add(
                            QT_sb[:, c, j * NT:(j + 1) * NT], psl[j][:],
                            BCOL_sb[:, c:c + 1])

                v_half(1)
                # V unpacks last on the SWDGE queue (PV needs them latest)
                for h in range(2):
                    for b in range(2):
                        nc.gpsimd.dma_start(
                            V_sb[:, h, b * DC:(b + 1) * DC, :], v_out[h][b])

            # ---- phase B: scores -> softmax -> +I^T -> attn^T ---------
            # The per-si epilogue (exp -> den -> diag -> transposes -> +I^T)
            # is software-pipelined one si behind the scores matmuls so the
            # PE never waits on the ACT/DVE chain. Transposes are batched 4
            # per PSUM bank so each intensity-add is one 512-wide DVE op.
            with (
                tc.tile_pool(name="e_pool", bufs=2) as e_pool,
                tc.tile_pool(name="stat_pool", bufs=2) as stat_pool,
            ):
                def si_scores(si):
                    E_sb = e_pool.tile([P, S], bf16, tag="e")
                    psl = [mm_ps.tile([P, NT], f32, tag="mm", name="ps")
                           for _ in range(TJ)]
                    for i in range(NDR):
                        for u in range(TJ):
                            nc.tensor.matmul(
                                psl[u][:],
                                QT_sb[:, 2 * i:2 * i + 2,
                                      si * P:(si + 1) * P],
                                KT_sb[:, u // 2, u % 2, 2 * i:2 * i + 2, :],
                                start=(i == 0), stop=(i == NDR - 1),
                                perf_mode=DR,
                            )
                    for u in range(TJ):
                        nc.scalar.activation(
                            E_sb[:, u * NT:(u + 1) * NT], psl[u][:], Exp,
                            scale=SCALE8, accum_out=ACC_sb[:, si, u:u + 1],
                        )
                    den = stat_pool.tile([P, 1], f32, tag="den")
                    recip = stat_pool.tile([P, 1], f32, tag="recip")
                    diag = stat_pool.tile([P, P], bf16, tag="diag")
                    nc.vector.reduce_sum(
                        den[:], ACC_sb[:, si, :], axis=mybir.AxisListType.X)
                    nc.vector.reciprocal(recip[:], den[:])
                    # diag(recip): identity rows scaled per-partition
                    nc.vector.tensor_scalar_mul(diag[:], ident[:], recip[:])
                    return E_sb, diag

                def si_transposes(si, E_sb, diag):
                    # attn^T = E-slice.T @ diag (transpose + normalize), 4
                    # tiles per PSUM bank; PSUM drain adds intensity^T.
                    for t4 in range(TJ):
                        pt = tr_ps.tile([P, NT], f32, tag="tr")
                        for q in range(4):
                            t = t4 * 4 + q
                            nc.tensor.matmul(
                                pt[:, q * P:(q + 1) * P],
                                E_sb[:, t * P:(t + 1) * P], diag[:],
                                start=True, stop=True,
                                skip_group_check=True)
                        nc.vector.tensor_tensor(
                            AT_sb[:, t4 * 4:(t4 + 1) * 4,
                                  si * P:(si + 1) * P],
                            pt[:], IT_sb[:, si, t4 * NT:(t4 + 1) * NT], add)

                prev = None
                for si in range(DC):
                    cur = si_scores(si)
                    if prev is not None:
                        si_transposes(si - 1, *prev)
                    prev = cur
                si_transposes(DC - 1, *prev)

            # ---- phase D/E: PV -> out^T, then projection per s-tile ---
            OT_sb = persist.tile([P, DC, SH], bf16, tag="qt_ot")
            with (
                tc.tile_pool(name="wo_pool", bufs=1) as wo_pool,
                tc.tile_pool(name="fin_pool", bufs=3) as fin_pool,
            ):
                WO_sb = wo_pool.tile([P, DC, D], bf16)
                nc.sync.dma_start(WO_sb[:], WO_d[:])
                for sj in range(SJ):
                    for dvi in range(DC):
                        ps = mm_ps.tile([P, NT], f32, tag="mm")
                        for t in range(TC):
                            nc.tensor.matmul(
                                ps[:],
                                V_sb[:, dvi // 4, t,
                                     (dvi % 4) * P:(dvi % 4 + 1) * P],
                                AT_sb[:, t, sj * NT:(sj + 1) * NT],
                                start=(t == 0),
                                stop=False,
                            )
                        # bias: bv (x) (1 + rowsum(I))
                        nc.tensor.matmul(
                            ps[:], BV_sb[0:1, dvi * P:(dvi + 1) * P],
                            RS_sb[0:1, sj * NT:(sj + 1) * NT],
                            start=False, stop=True)
                        nc.vector.tensor_copy(
                            out=OT_sb[:, dvi, sj * NT:(sj + 1) * NT], in_=ps[:])

                    for doi in range(DC):
                        ps = mm_ps.tile([P, NT], f32, tag="mm")
                        for dvc in range(DC):
                            nc.tensor.matmul(
                                ps[:],
                                WO_sb[:, dvc, doi * P:(doi + 1) * P],
                                OT_sb[:, dvc, sj * NT:(sj + 1) * NT],
                                start=(dvc == 0), stop=(dvc == DC - 1),
                            )
                        F_sb = fin_pool.tile([P, NT], f32, tag="fin")
                        nc.vector.tensor_scalar_add(
                            F_sb[:], ps[:],
                            BCOL_sb[:, 2 * DC + doi:2 * DC + doi + 1])
                        nc.sync.dma_start(
                            out_v[:, doi, sj * NT:(sj + 1) * NT], F_sb[:])

    nc.compile()
    return nc


def _get_module():
    if "nc" not in _CACHE:
        _CACHE["nc"] = _build_module()
    return _CACHE["nc"]


def _chunked(a, ncols):
    """[D, ncols] -> [P, DC, ncols] with partition p holding rows c*128+p."""
    return np.ascontiguousarray(
        a.reshape(DC, P, ncols).transpose(1, 0, 2))


def _make_in_maps(inputs):
    X = np.asarray(inputs["X"], dtype=np.float32)
    intensity = np.asarray(inputs["intensity"], dtype=np.float32)
    bf = ml_dtypes.bfloat16
    f8 = ml_dtypes.float8_e4m3
    Wq8 = _chunked(np.ascontiguousarray(
        np.asarray(inputs["Wq"], np.float32).T * 32.0), D).astype(f8)
    Wk8 = _chunked(np.ascontiguousarray(
        np.asarray(inputs["Wk"], np.float32).T * 32.0), D).astype(f8)
    WvT = _chunked(np.ascontiguousarray(
        np.asarray(inputs["Wv"], np.float32).T), D).astype(bf)
    WoT = _chunked(np.ascontiguousarray(
        np.asarray(inputs["Wo"], np.float32).T), D).astype(bf)
    bq, bk, bv, bo = (np.asarray(inputs[k], np.float32).reshape(D)
                      for k in ("bq", "bk", "bv", "bo"))
    BCOL = np.concatenate(
        [b.reshape(DC, P).T for b in (32.0 * bq, 32.0 * bk, bo)], axis=1
    ).astype(np.float32)  # [128, 24]

    in_maps = []
    for c in range(8):
        b, h = c // 2, c % 2
        XT = np.ascontiguousarray(X[b, h * SH:(h + 1) * SH, :].T)
        XQT = _chunked(XT, SH).astype(bf)
        X8T = _chunked(XT, SH).astype(f8)
        Islc = intensity[b, h * SH:(h + 1) * SH, :]
        # [t, s] -> [si*128+tp, tc*128+sp] so each per-si load is one
        # contiguous row-block (128 descriptors instead of 2048)
        IT = np.ascontiguousarray(
            Islc.T.reshape(TC, P, DC, P).transpose(2, 1, 0, 3).reshape(SH, S)
        ).astype(bf)
        rows = 1.0 + Islc.sum(axis=1, dtype=np.float64).astype(np.float32)
        BROW = np.concatenate([bv, rows]).reshape(1, D + SH)
        in_maps.append({
            "X8T": X8T, "XQT": XQT, "WQ8": Wq8, "WK8": Wk8,
            "WVT": WvT, "WOT": WoT,
            "BCOL": BCOL, "BROW": BROW, "IT": IT,
        })
    return in_maps


def _gather(results):
    out = np.empty((4, S, D), dtype=np.float32)
    for c in range(8):
        b, h = c // 2, c % 2
        out[b, h * SH:(h + 1) * SH, :] = results[c]["OUTT"].T
    return out


def kernel(**inputs):
    from concourse import bass_utils

    in_maps = _make_in_maps(inputs)
    nc = _get_module()
    res = bass_utils.run_bass_kernel_spmd(nc, in_maps, core_ids=list(range(8)))
    return _gather(res.results)


# revision 98
# speedup vs baseline: 1.2099x; 1.0168x over previous
"""Trainium2 Bass kernel for nn_Attention_54254026883778.

Single-head attention with an additive post-softmax intensity term:
    q/k/v = X @ W{q,k,v}.T + b;  scores = q k^T / sqrt(D)
    attn  = softmax(scores) + intensity;  out = (attn @ v) @ Wo.T + bo

Sharding: 8 cores = 4 batches x 2 sequence halves. Each core computes
K^T and V for its own 1024 rows; the partner half arrives via 2-rank
AllGathers through shared DRAM (global t-order, SPMD-static).

Precision plan: Q/K projections and the scores matmul run in fp8e4m3
with DoubleRow (2x PE throughput). Host ships X and 32*W{q,k}^T in fp8
(the x32 shift keeps the tiny weights in fp8's normal range); the exp
activation folds the 1/(32*32) back in on top of 1/sqrt(D). Softmax
errors are ~1000x attenuated in the output (softmax rows sum to 1 while
the additive intensity rows sum to ~1024), so fp8 there is safe. The
V projection, PV and output projection stay bf16.

Dataflow per core (own 1024 query rows, full 2048 keys):
    warmup   dummy matmuls under the initial DMA so the PE HAM
             clock-gate opens before real work arrives
    K^T      [dout | t-own]  fp8 DR, j-outer; per-j fp8 AllGather
    V        [t-own | dv]    bf16, dv-half-outer; per-half AllGather
    Q^T      [dout | s]      fp8 DR
    scores   [s | t]         fp8 DR -> exp on ACT with fused
        row-accumulate -> 1/den on DVE -> diag(recip) ->
        attn^T tile = E-slice.T @ diag(recip)  (PE transposes+normalizes)
        -> DVE adds intensity^T while draining PSUM -> attn^T [t | s]
    out^T    [dv | s]    = V-chunk.T @ attn^T   (bf16)
    final^T  [do | s]    = WoT-chunk.T @ out^T  -> DRAM, host transposes
Biases: q/k (x32) and o enter as per-partition adds during PSUM extract;
v enters as a rank-1 fp32r matmul bv (x) (1 + rowsum(I)).

DMA plan (the previous round stalled 50us on queue head-blocking):
  - big inputs are host-swizzled to [P, chunks, cols] so each is ONE
    trigger with 8-16KB contiguous per-partition packets
  - all collective staging/unpacks ride the SWDGE (gpsimd) queue in an
    emission order that never parks an unpack in front of a stage
  - intensity loads + WO + outputs stay on the sync HWDGE queue
"""

import numpy as np
import ml_dtypes

P = 128
D = 1024
S = 2048          # keys per batch (full sequence)
SH = 1024         # query rows owned by each core
DC = D // P       # 8  contraction chunks over model dim
TC = S // P       # 16 t (key) chunks
NT = 512          # matmul moving free dim / psum bank
SJ = SH // NT     # 2  s-tiles of own rows
TJ = S // NT      # 4  t-tiles
SCALE = 1.0 / 32.0        # 1/sqrt(D)
SCALE8 = SCALE / 1024.0   # undo the x32 on q and k

_CACHE = {}


def _build_module():
    import concourse.bass as bass
    import concourse.tile as tile
    import concourse.mybir as mybir
    from concourse import bacc
    from concourse.masks import make_identity

    f32 = mybir.dt.float32
    f32r = mybir.dt.float32r
    bf16 = mybir.dt.bfloat16
    fp8 = mybir.dt.float8e4
    DR = mybir.MatmulPerfMode.DoubleRow
    Exp = mybir.ActivationFunctionType.Exp
    add = mybir.AluOpType.add

    nc = bacc.Bacc("TRN2", target_bir_lowering=False, debug=False,
                   num_devices=8)

    X8_d = nc.dram_tensor("X8T", [P, DC, SH], fp8, kind="ExternalInput")
    XQ_d = nc.dram_tensor("XQT", [P, DC, SH], bf16, kind="ExternalInput")
    WQ_d = nc.dram_tensor("WQ8", [P, DC, D], fp8, kind="ExternalInput")
    WK_d = nc.dram_tensor("WK8", [P, DC, D], fp8, kind="ExternalInput")
    WV_d = nc.dram_tensor("WVT", [P, DC, D], bf16, kind="ExternalInput")
    WO_d = nc.dram_tensor("WOT", [P, DC, D], bf16, kind="ExternalInput")
    BCOL_d = nc.dram_tensor("BCOL", [P, 3 * DC], f32, kind="ExternalInput")
    BROW_d = nc.dram_tensor("BROW", [1, D + SH], f32, kind="ExternalInput")
    IT_d = nc.dram_tensor("IT", [SH, S], bf16, kind="ExternalInput")
    OUT_d = nc.dram_tensor("OUTT", [D, SH], f32, kind="ExternalOutput")

    out_v = OUT_d[:].rearrange("(c p) s -> p c s", p=P)

    GROUPS = [[0, 1], [2, 3], [4, 5], [6, 7]]
    NDR = DC // 2  # 4 contraction pair-chunks for DoubleRow

    with tile.TileContext(nc) as tc:
        with (
            tc.tile_pool(name="persist", bufs=1) as persist,
            tc.tile_pool(name="mm_ps", bufs=6, space="PSUM") as mm_ps,
            tc.tile_pool(name="tr_ps", bufs=2, space="PSUM") as tr_ps,
            tc.tile_pool(name="dram", bufs=1, space="DRAM") as dram_pool,
        ):
            # ---- persistent tiles -------------------------------------
            KT_sb = persist.tile([P, 2, SJ, DC, NT], fp8)   # K^T gathered
            V_sb = persist.tile([P, 2, TC, NT], bf16)       # V [t | half,dv]
            QT_sb = persist.tile([P, DC, SH], fp8, tag="qt_ot")
            X8_sb = persist.tile([P, DC, SH], fp8)          # X^T own, fp8
            XQ_sb = persist.tile([P, DC, SH], bf16, tag="xq_at")
            KL_sb = persist.tile([P, SJ, DC, NT], fp8)      # K^T local (x32)
            VL_sb = persist.tile([P, SJ, DC, NT], bf16)     # local V halves
            AT_sb = persist.tile([P, TC, SH], bf16, tag="xq_at")  # attn^T
            ACC_sb = persist.tile([P, DC, TJ], f32)
            IT_sb = persist.tile([P, DC, S], bf16)          # intensity^T

            ident = persist.tile([P, P], bf16)
            make_identity(nc, ident)
            WARM_sb = persist.tile([P, NT], bf16)
            nc.vector.memset(WARM_sb[:], 0.0)
            # bq*32|bk*32|bo as per-partition columns, added on PSUM extract
            BCOL_sb = persist.tile([P, 3 * DC], f32)
            nc.sync.dma_start(BCOL_sb[:], BCOL_d[:])
            # bv and the attn rowsums feed the PV rank-1 bias matmul; fp32r
            # operands must come from a rounding instruction, so stage the
            # DMA through a DVE copy.
            BROW_r = persist.tile([1, D + SH], f32r)
            with tc.tile_pool(name="brow_pool", bufs=1) as brow_pool:
                BROW_ld = brow_pool.tile([1, D + SH], f32)
                nc.sync.dma_start(BROW_ld[:], BROW_d[:])
                nc.vector.tensor_copy(out=BROW_r[:], in_=BROW_ld[:])
            BV_sb = BROW_r[0:1, 0:D]
            RS_sb = BROW_r[0:1, D:D + SH]                   # 1 + rowsum(I)

            # ---- PE warmup: dummy matmuls under the initial DMA -------
            for _ in range(20):
                wps = mm_ps.tile([P, NT], f32, tag="mm", name="warm")
                nc.tensor.matmul(wps[:], ident[:], WARM_sb[:],
                                 start=True, stop=True)


            # collective DRAM tiles: K per j-half (fp8), V per dv-half
            k_in = [dram_pool.tile([P, DC, NT], fp8, name=f"k_in{j}")
                    for j in range(SJ)]
            k_out = [dram_pool.tile([2, P, DC, NT], fp8, name=f"k_out{j}")
                     for j in range(SJ)]
            v_in = [dram_pool.tile([P, DC, NT], bf16, name=f"v_in{h}")
                    for h in range(2)]
            v_out = [dram_pool.tile([2, P, DC, NT], bf16, name=f"v_out{h}")
                     for h in range(2)]

            # ---- phase A: K full-local, V-local/gather, Q -------------
            with (
                tc.tile_pool(name="w8pool", bufs=1) as w8pool,
                tc.tile_pool(name="wvpool", bufs=1) as wvpool,
            ):
                # All inputs ride the sync queue in deadline order (the
                # proven config); WQ8 shares the single w8pool buffer with
                # WK8 so its load is WAR-gated until K consumed the weights.
                WK_sb = w8pool.tile([P, DC, D], fp8, tag="w8")
                nc.sync.dma_start(X8_sb[:], X8_d[:])
                nc.sync.dma_start(WK_sb[:], WK_d[:])
                WV_sb = wvpool.tile([P, DC, D], bf16)
                nc.sync.dma_start(XQ_sb[:], XQ_d[:])
                nc.sync.dma_start(WV_sb[:], WV_d[:])
                # intensity prefetch on the otherwise-empty SCALAR queue:
                # keeps its 4MB off the sync queue, so the K unpacks (the
                # final leg of the collective wall, ~85us) and the WQ8 load
                # are not slowed by it. Loaded as 8 per-si slices whose
                # small 4KB packets trickle at ~25% of HBM, so the startup
                # X8/WK8 loads keep most of the bandwidth.
                it_v = IT_d[:].rearrange("(si p) f -> si p f", p=P)
                for si in range(DC):
                    nc.scalar.dma_start(IT_sb[:, si, :], it_v[si])

                # K^T local [dout, t-own] fp8 DoubleRow, j-outer; stage and
                # gather each j-half on the SWDGE as soon as it is done so
                # both doorbells ring by ~36us and the wire work finishes
                # well before the Q projection does.
                for j in range(SJ):
                    for c in range(DC):
                        ps = mm_ps.tile([P, NT], f32, tag="mm", name="ps")
                        for i in range(NDR):
                            nc.tensor.matmul(
                                ps[:],
                                WK_sb[:, 2 * i:2 * i + 2, c * P:(c + 1) * P],
                                X8_sb[:, 2 * i:2 * i + 2,
                                      j * NT:(j + 1) * NT],
                                start=(i == 0), stop=(i == NDR - 1),
                                perf_mode=DR,
                            )
                        nc.vector.tensor_scalar_add(
                            KL_sb[:, j, c, :], ps[:],
                            BCOL_sb[:, DC + c:DC + c + 1])
                    nc.gpsimd.dma_start(k_in[j][:], KL_sb[:, j])
                    nc.gpsimd.collective_compute(
                        "AllGather", mybir.AluOpType.bypass,
                        replica_groups=GROUPS,
                        ins=[k_in[j].opt()], outs=[k_out[j].opt()])

                # V local, dv-half-outer (h = output half); VL is [P,h,t,dv]
                # so each half stages as one contiguous 8KB/partition DMA.
                # The SECOND half runs AFTER the Q projection: it fills the
                # PE while the K AllGather results land, so the scores never
                # wait on the collective chain (and either order of {V-h1,
                # Q} keeps that property if the scheduler swaps them).
                def v_half(h):
                    for t in range(DC):
                        ps = mm_ps.tile([P, NT], f32, tag="mm", name="ps")
                        for dc in range(DC):
                            nc.tensor.matmul(
                                ps[:],
                                XQ_sb[:, dc, t * P:(t + 1) * P],
                                WV_sb[:, dc, h * NT:(h + 1) * NT],
                                start=(dc == 0),
                                stop=(dc == DC - 1),
                            )
                        nc.vector.tensor_copy(
                            out=VL_sb[:, h, t, :], in_=ps[:])
                    nc.gpsimd.dma_start(v_in[h][:], VL_sb[:, h])
                    nc.gpsimd.collective_compute(
                        "AllGather", mybir.AluOpType.bypass,
                        replica_groups=GROUPS,
                        ins=[v_in[h].opt()], outs=[v_out[h].opt()])

                v_half(0)

                # Q^T [dout, s-own] fp8 DoubleRow; WQ8 reuses WK8's buffer
                # (bufs=1 WAR-gates its DMA behind the last K matmul)
                WQ_sb = w8pool.tile([P, DC, D], fp8, tag="w8")
                nc.sync.dma_start(WQ_sb[:], WQ_d[:])
                # K unpacks on the sync HWDGE queue, emitted after every
                # input trigger so their ccK semaphore waits can only
                # head-block the phase-D WO/output triggers (far later)
                for j in range(SJ):
                    for b in range(2):
                        nc.sync.dma_start(KT_sb[:, b, j], k_out[j][b])
                for c in range(DC):
                    psl = [mm_ps.tile([P, NT], f32, tag="mm", name="ps")
                           for _ in range(SJ)]
                    for i in range(NDR):
                        for j in range(SJ):
                            nc.tensor.matmul(
                                psl[j][:],
                                WQ_sb[:, 2 * i:2 * i + 2, c * P:(c + 1) * P],
                                X8_sb[:, 2 * i:2 * i + 2,
                                      j * NT:(j + 1) * NT],
                                start=(i == 0), stop=(i == NDR - 1),
                                perf_mode=DR,
                            )
                    for j in range(SJ):
                        nc.vector.tensor_scalar_add(
                            QT_sb[:, c, j * NT:(j + 1) * NT], psl[j][:],
                            BCOL_sb[:, c:c + 1])

                v_half(1)
                # V unpacks last on the SWDGE queue (PV needs them latest)
                for h in range(2):
                    for b in range(2):
                        nc.gpsimd.dma_start(
                            V_sb[:, h, b * DC:(b + 1) * DC, :], v_out[h][b])

            # ---- phase B: scores -> softmax -> +I^T -> attn^T ---------
            # The per-si epilogue (exp -> den -> diag -> transposes -> +I^T)
            # is software-pipelined one si behind the scores matmuls so the
            # PE never waits on the ACT/DVE chain. Transposes are batched 4
            # per PSUM bank so each intensity-add is one 512-wide DVE op.
            with (
                tc.tile_pool(name="e_pool", bufs=2) as e_pool,
                tc.tile_pool(name="stat_pool", bufs=2) as stat_pool,
            ):
                def si_scores(si):
                    E_sb = e_pool.tile([P, S], bf16, tag="e")
                    psl = [mm_ps.tile([P, NT], f32, tag="mm", name="ps")
                           for _ in range(TJ)]
                    for i in range(NDR):
                        for u in range(TJ):
                            nc.tensor.matmul(
                                psl[u][:],
                                QT_sb[:, 2 * i:2 * i + 2,
                                      si * P:(si + 1) * P],
                                KT_sb[:, u // 2, u % 2, 2 * i:2 * i + 2, :],
                                start=(i == 0), stop=(i == NDR - 1),
                                perf_mode=DR,
                            )
                    for u in range(TJ):
                        nc.scalar.activation(
                            E_sb[:, u * NT:(u + 1) * NT], psl[u][:], Exp,
                            scale=SCALE8, accum_out=ACC_sb[:, si, u:u + 1],
                        )
                    den = stat_pool.tile([P, 1], f32, tag="den")
                    recip = stat_pool.tile([P, 1], f32, tag="recip")
                    diag = stat_pool.tile([P, P], bf16, tag="diag")
                    nc.vector.reduce_sum(
                        den[:], ACC_sb[:, si, :], axis=mybir.AxisListType.X)
                    nc.vector.reciprocal(recip[:], den[:])
                    # diag(recip): identity rows scaled per-partition
                    nc.vector.tensor_scalar_mul(diag[:], ident[:], recip[:])
                    return E_sb, diag

                def si_transposes(si, E_sb, diag):
                    # attn^T = E-slice.T @ diag (transpose + normalize), 4
                    # tiles per PSUM bank; PSUM drain adds intensity^T.
                    for t4 in range(TJ):
                        pt = tr_ps.tile([P, NT], f32, tag="tr")
                        for q in range(4):
                            t = t4 * 4 + q
                            nc.tensor.matmul(
                                pt[:, q * P:(q + 1) * P],
                                E_sb[:, t * P:(t + 1) * P], diag[:],
                                start=True, stop=True,
                                skip_group_check=True)
                        nc.vector.tensor_tensor(
                            AT_sb[:, t4 * 4:(t4 + 1) * 4,
                                  si * P:(si + 1) * P],
                            pt[:], IT_sb[:, si, t4 * NT:(t4 + 1) * NT], add)

                prev = None
                for si in range(DC):
                    cur = si_scores(si)
                    if prev is not None:
                        si_transposes(si - 1, *prev)
                    prev = cur
                si_transposes(DC - 1, *prev)

            # ---- phase D/E: PV -> out^T, then projection per s-tile ---
            OT_sb = persist.tile([P, DC, SH], bf16, tag="qt_ot")
            with (
                tc.tile_pool(name="wo_pool", bufs=1) as wo_pool,
                tc.tile_pool(name="fin_pool", bufs=3) as fin_pool,
            ):
                WO_sb = wo_pool.tile([P, DC, D], bf16)
                nc.sync.dma_start(WO_sb[:], WO_d[:])
                for sj in range(SJ):
                    for dvi in range(DC):
                        ps = mm_ps.tile([P, NT], f32, tag="mm")
                        for t in range(TC):
                            nc.tensor.matmul(
                                ps[:],
                                V_sb[:, dvi // 4, t,
                                     (dvi % 4) * P:(dvi % 4 + 1) * P],
                                AT_sb[:, t, sj * NT:(sj + 1) * NT],
                                start=(t == 0),
                                stop=False,
                            )
                        # bias: bv (x) (1 + rowsum(I))
                        nc.tensor.matmul(
                            ps[:], BV_sb[0:1, dvi * P:(dvi + 1) * P],
                            RS_sb[0:1, sj * NT:(sj + 1) * NT],
                            start=False, stop=True)
                        nc.vector.tensor_copy(
                            out=OT_sb[:, dvi, sj * NT:(sj + 1) * NT], in_=ps[:])

                    for doi in range(DC):
                        ps = mm_ps.tile([P, NT], f32, tag="mm")
                        for dvc in range(DC):
                            nc.tensor.matmul(
                                ps[:],
                                WO_sb[:, dvc, doi * P:(doi + 1) * P],
                                OT_sb[:, dvc, sj * NT:(sj + 1) * NT],
                                start=(dvc == 0), stop=(dvc == DC - 1),
                            )
                        F_sb = fin_pool.tile([P, NT], f32, tag="fin")
                        nc.vector.tensor_scalar_add(
                            F_sb[:], ps[:],
                            BCOL_sb[:, 2 * DC + doi:2 * DC + doi + 1])
                        nc.sync.dma_start(
                            out_v[:, doi, sj * NT:(sj + 1) * NT], F_sb[:])

    nc.compile()
    return nc


def _get_module():
    if "nc" not in _CACHE:
        _CACHE["nc"] = _build_module()
    return _CACHE["nc"]


def _chunked(a, ncols):
    """[D, ncols] -> [P, DC, ncols] with partition p holding rows c*128+p."""
    return np.ascontiguousarray(
        a.reshape(DC, P, ncols).transpose(1, 0, 2))


def _make_in_maps(inputs):
    X = np.asarray(inputs["X"], dtype=np.float32)
    intensity = np.asarray(inputs["intensity"], dtype=np.float32)
    bf = ml_dtypes.bfloat16
    f8 = ml_dtypes.float8_e4m3
    Wq8 = _chunked(np.ascontiguousarray(
        np.asarray(inputs["Wq"], np.float32).T * 32.0), D).astype(f8)
    Wk8 = _chunked(np.ascontiguousarray(
        np.asarray(inputs["Wk"], np.float32).T * 32.0), D).astype(f8)
    WvT = _chunked(np.ascontiguousarray(
        np.asarray(inputs["Wv"], np.float32).T), D).astype(bf)
    WoT = _chunked(np.ascontiguousarray(
        np.asarray(inputs["Wo"], np.float32).T), D).astype(bf)
    bq, bk, bv, bo = (np.asarray(inputs[k], np.float32).reshape(D)
                      for k in ("bq", "bk", "bv", "bo"))
    BCOL = np.concatenate(
        [b.reshape(DC, P).T for b in (32.0 * bq, 32.0 * bk, bo)], axis=1
    ).astype(np.float32)  # [128, 24]

    in_maps = []
    for c in range(8):
        b, h = c // 2, c % 2
        XT = np.ascontiguousarray(X[b, h * SH:(h + 1) * SH, :].T)
        XQT = _chunked(XT, SH).astype(bf)
        X8T = _chunked(XT, SH).astype(f8)
        Islc = intensity[b, h * SH:(h + 1) * SH, :]
        # [t, s] -> [si*128+tp, tc*128+sp] so each per-si load is one
        # contiguous row-block (128 descriptors instead of 2048)
        IT = np.ascontiguousarray(
            Islc.T.reshape(TC, P, DC, P).transpose(2, 1, 0, 3).reshape(SH, S)
        ).astype(bf)
        rows = 1.0 + Islc.sum(axis=1, dtype=np.float64).astype(np.float32)
        BROW = np.concatenate([bv, rows]).reshape(1, D + SH)
        in_maps.append({
            "X8T": X8T, "XQT": XQT, "WQ8": Wq8, "WK8": Wk8,
            "WVT": WvT, "WOT": WoT,
            "BCOL": BCOL, "BROW": BROW, "IT": IT,
        })
    return in_maps


def _gather(results):
    out = np.empty((4, S, D), dtype=np.float32)
    for c in range(8):
        b, h = c // 2, c % 2
        out[b, h * SH:(h + 1) * SH, :] = results[c]["OUTT"].T
    return out


def kernel(**inputs):
    from concourse import bass_utils

    in_maps = _make_in_maps(inputs)
    nc = _get_module()
    res = bass_utils.run_bass_kernel_spmd(nc, in_maps, core_ids=list(range(8)))
    return _gather(res.results)
